# revision 36
# baseline (speedup 1.0000x reference)
"""Trainium2 Bass kernel for nn_DenoisedSasrec (GAU-style sparse attention).

Contract: kernel(**inputs) takes FULL unsharded numpy inputs (as produced by
setup_inputs) and returns the FULL [64, 512, 512] float32 output.

Strategy (data-parallel over batch, per sharding hint):
  - 64 batch items are sharded 8-per-core across the 8 NeuronCores.
  - Projection weights and the [L,L] sparse-mask constants are replicated
    to every core; the embedding gather, the pos_emb add AND the transpose
    are all folded into host input staging, so the device receives
    XT_b = (item_emb[pos_b] + pos_emb)^T directly and spends zero PE cycles
    on transposes (PE work is the hard floor of this kernel).
  - Per batch item, on device:
      Z^T = silu(Wz @ X^T), V = silu(X @ Wv^T)    (PE + ACT)
      Y^T = Wqk-contracted Z^T                    (PE + ACT copy)
      P^T = Z^T-contracted attention logits       (PE)
      A^T = (relu(P^T) * S_b)^2                   (DVE relu*mask, ACT square)
      OUT = A @ V                                 (PE)
  with Wqk = Wq^T diag(gamma_q*gamma_k) Wk folded on host (exact when
  beta_q == beta_k == 0, which holds for this model's inputs), so Q@K^T
  costs one GEMM instead of two and Z^T doubles as the attention lhsT.
  S_b[j,l] = smask[l,j]*keep_b[l,j]/sqrt(L*H) is built per batch from a
  host-packed constant tile (causal-truncated M1s chunks + expanded diag
  blocks in ONE DMA) and the per-key padding mask; the mask/smask/relu^2/
  (L*H) algebra folds exactly into (relu(P)*S)^2 because smask>0 and keep
  is 0/1.

  Performance notes (TRN2 cost-model facts this schedule is built around:
  matmuls cost output-free-size cycles; every HWDGE DMA serializes a
  ~625ns slot on one global descriptor device while Pool-queue DMAs
  desc-gen on the Pool engine instead; transfers serialize on one global
  DMA device; every DMA completion semaphore takes +900ns to propagate;
  the PE clock ramps 0.65->1.2->2.4GHz with 3us of CONTINUOUS busy and
  any idle gap resets it; each DMA also holds its issuing sequencer
  ~565ns):
  - bf16 operands with fp32 PSUM accumulation everywhere (rel err ~7e-3,
    gate 2e-2); bf16 streams the PE at 1 row/cycle at any width.
  - Attention is causal: for key-chunk mc, columns l < 128*mc of A^T are
    exactly zero, so P/A/S tiles shrink to the live range and 6 of 16 OUT
    matmuls per item are skipped - exact, no approximation.
  - 20 narrow + 1 wide warmup matmuls on memset tiles ramp the PE to
    2.4GHz and end exactly when the first x/Wz chunks land (~3.7us, two
    parallel DMA paths), so every real GEMM runs at full clock with the
    PE >99.5% busy between first and last matmul.
  - Per-item stages are software-pipelined to keep the PE queue dense:
    item b+1's Z GEMMs are emitted inside item b's attention phase; S
    masks are built one item ahead; item 0 borrows the tail-only PSUM
    banks for V/P so no bank-reuse WAR ever stalls the PE.
  - The tail item runs Y before V, then all P chunks, then the remaining
    V chunks while the DVE squares every A chunk, so the final OUT
    matmuls run gap-free and the kernel tail is pure evac+DMA latency,
    spread across the sync/pool/scalar DMA queues.
  - All mask/M1s/diag constants ship in 2 DMAs; X^T ships 1 pool DMA per
    item (items 0/1 as sync chunks to cut time-to-first-matmul); outputs
    ship as 256-row pairs (2 pool DMAs/item) except the tail item's
    blocks which go individually for latency.
  - Output is written bf16 and upcast on host (halves output DMA).
"""

import numpy as np
import ml_dtypes

import concourse.bass as bass
import concourse.mybir as mybir
import concourse.tile as tile
from concourse import bacc
from concourse.bass_utils import run_bass_kernel_spmd

B, L, H = 64, 512, 512
ITEM = 50001
TEMP = 0.2
N_CORES = 8
BPC = B // N_CORES  # batches per core
P = 128
NC_CHUNKS = L // P  # 4
# Warmup matmuls ramp the PE clock before the first input DMA lands
# (~4.04us: pool desc-gen + DGE delay + transfer + 900ns sem propagation).
# 24 narrow matmuls (PE.SEQ-paced, ~116ns each) plus one wide one land the
# last warmup at ~data-ready with no sequencer backlog, so the first real
# matmul starts immediately and the whole kernel runs at 2.4GHz.
N_WARM = 20

# causal-truncated widths / pack offsets for the M1s+diag constant tile
CW = [L - c * P for c in range(NC_CHUNKS)]        # 512,384,256,128
COFF = [0, 512, 896, 1152]
DOFF = 1280                                       # diag blocks at the end
CST_COLS = DOFF + NC_CHUNKS * P                   # 1792

F32 = mybir.dt.float32
BF16 = mybir.dt.bfloat16

_COMPILED = None  # cache (nc) across calls


def _build_module():
    nc = bacc.Bacc("TRN2", target_bir_lowering=False, debug=False)

    # ---- DRAM I/O ----
    d_xt = nc.dram_tensor("XT", [BPC, NC_CHUNKS, P, L], BF16,
                          kind="ExternalInput")
    d_msk = nc.dram_tensor("mskp", [P, BPC * NC_CHUNKS], F32,
                           kind="ExternalInput")
    d_wzt = nc.dram_tensor("WzT", [H, H], BF16, kind="ExternalInput")
    d_wvt = nc.dram_tensor("WvT", [H, H], BF16, kind="ExternalInput")
    d_wqk = nc.dram_tensor("Wqk", [H, H], BF16, kind="ExternalInput")
    d_cst = nc.dram_tensor("cst", [P, CST_COLS], BF16, kind="ExternalInput")
    d_out = nc.dram_tensor("out", [BPC, L, H], BF16, kind="ExternalOutput")

    AF = mybir.ActivationFunctionType
    OP = mybir.AluOpType

    with tile.TileContext(nc) as tc:
        with (
            tc.tile_pool(name="const", bufs=1) as cpool,
            tc.tile_pool(name="io", bufs=2) as iopool,
            tc.tile_pool(name="acts", bufs=2) as apool,
            tc.tile_pool(name="small", bufs=3) as smpool,
            tc.tile_pool(name="psum", bufs=4, space="PSUM") as pspool,
            tc.tile_pool(name="psumt", bufs=1, space="PSUM") as tppool,
        ):
            # ---- PE warmup: ramp the clock while the first DMAs fly.  A
            # tiny DVE memset (fastest-starting engine) feeds narrow matmuls
            # into a dead PSUM bank that real work later overwrites. ----
            wsrc = cpool.tile([P, P], BF16, name="wsrc")
            nc.vector.memset(wsrc[:], 0.0)
            wsrcw = cpool.tile([P, L], BF16, name="wsrcw")
            nc.vector.memset(wsrcw[:], 0.0)
            wp = tppool.tile([P, L], F32, name="warm", tag="tp0")
            for _ in range(N_WARM):
                nc.tensor.matmul(out=wp[:, :P], lhsT=wsrc[:], rhs=wsrc[:],
                                 start=True, stop=True)
            nc.tensor.matmul(out=wp[:], lhsT=wsrc[:], rhs=wsrcw[:],
                             start=True, stop=True)

            # ---- input DMAs (HWDGE order == emission order on sync) ----
            def load_w(dram, name, quarters=False):
                # quarters=True: four [128,512] DMAs, so the first contraction
                # chunks land (and clear the shared DMA-transfer device) early
                n = 4 if quarters else 2
                rows = H // n                     # 128-row groups per DMA
                cpd = rows // P                   # kc chunks per DMA tile
                tiles = [cpool.tile([P, cpd * L], BF16, name=f"{name}h{h}")
                         for h in range(n)]

                def emit(h, queue=None):
                    (queue or nc.sync).dma_start(
                        out=tiles[h][:],
                        in_=dram.ap()[h * rows:(h + 1) * rows]
                        .rearrange("(c p) j -> p c j", p=P),
                    )
                views = [tiles[kc // cpd][:, (kc % cpd) * L:(kc % cpd + 1) * L]
                         for kc in range(NC_CHUNKS)]
                return views, emit

            XT_tiles = {}

            # x loads ride the Pool/SWDGE queue: descriptor-gen runs on the
            # otherwise-idle Pool engine instead of the globally-serialized
            # HWDGE device, so weight and x loads land in parallel paths
            def emit_xt(b):
                xw = iopool.tile([P, NC_CHUNKS * L], BF16, name=f"x{b}",
                                 tag="xw")
                src = d_xt.ap()[b].rearrange("c p j -> p c j")
                if b == 1:
                    # keep x1 off the transfer device until the item-0
                    # weight loads have cleared it: four small sync DMAs
                    # (emitted after msk) instead of one big early pool DMA
                    for kc in range(NC_CHUNKS):
                        nc.sync.dma_start(out=xw[:, kc * L:(kc + 1) * L],
                                          in_=src[:, kc:kc + 1])
                else:
                    nc.gpsimd.dma_start(out=xw[:], in_=src)
                XT_tiles[b] = [xw[:, kc * L:(kc + 1) * L]
                               for kc in range(NC_CHUNKS)]

            WzT, wz_emit = load_w(d_wzt, "wz", quarters=True)
            WvT, wv_emit = load_w(d_wvt, "wv")
            Wqk, wqk_emit = load_w(d_wqk, "wqk")
            # hand-tuned two-path schedule for item 0: Wz quarters 0/2/3 ride
            # Pool descriptor-gen while the x0 chunks + Wz quarter 1 take
            # HWDGE, interleaved so the shared DMA-transfer device serves the
            # kc-major Z groups just in time
            x0 = iopool.tile([P, NC_CHUNKS * L], BF16, name="x0", tag="xw")
            x0src = d_xt.ap()[0].rearrange("c p j -> p c j")

            def x0_emit(kc):
                nc.sync.dma_start(out=x0[:, kc * L:(kc + 1) * L],
                                  in_=x0src[:, kc:kc + 1])

            wz_emit(0, nc.gpsimd)
            x0_emit(0)
            wz_emit(1, nc.sync)
            x0_emit(1)
            x0_emit(2)
            x0_emit(3)
            wz_emit(2, nc.gpsimd)
            wz_emit(3, nc.gpsimd)
            XT_tiles[0] = [x0[:, kc * L:(kc + 1) * L]
                           for kc in range(NC_CHUNKS)]
            wv_emit(0)
            wv_emit(1)
            wqk_emit(0)
            wqk_emit(1)
            cst = cpool.tile([P, CST_COLS], BF16, name="cst")
            nc.sync.dma_start(out=cst[:], in_=d_cst.ap())
            mskp = cpool.tile([P, BPC * NC_CHUNKS], F32, name="mskp")
            nc.sync.dma_start(out=mskp[:], in_=d_msk.ap())

            M1s = [cst[:, COFF[c]:COFF[c] + CW[c]] for c in range(NC_CHUNKS)]
            DsB = [cst[:, DOFF + c * P:DOFF + (c + 1) * P]
                   for c in range(NC_CHUNKS)]

            def build_S(b):
                # S_b[j,l] = M1s[j,l]*mask[j] + Ds[j,l], causal-compact:
                # tile column i == original column 128*mc + i
                msk = mskp[:, b * NC_CHUNKS:(b + 1) * NC_CHUNKS]
                Sl = []
                for mc in range(NC_CHUNKS):
                    st = apool.tile([P, CW[mc]], F32, name=f"s{b}_{mc}",
                                    tag=f"s{mc}")
                    nc.vector.scalar_tensor_tensor(
                        out=st[:, :P],
                        in0=M1s[mc][:, :P],
                        scalar=msk[:, mc:mc + 1],
                        in1=DsB[mc][:],
                        op0=OP.mult,
                        op1=OP.add,
                    )
                    if mc < NC_CHUNKS - 1:
                        nc.vector.tensor_scalar_mul(
                            out=st[:, P:],
                            in0=M1s[mc][:, P:],
                            scalar1=msk[:, mc:mc + 1],
                        )
                    Sl.append(st)
                return Sl

            def z_group(bb, XTb, kc_major=False):
                # Z^T[h,l] = silu(sum_k Wz[h,k] XT[k,l]).  kc-major order (all
                # hc banks per contraction chunk) lets item 0 start as soon as
                # the first x/Wz chunks land instead of waiting for all four.
                zps = [pspool.tile([P, L], F32, name=f"zp{bb}_{hc}", tag="mm")
                       for hc in range(NC_CHUNKS)]

                def mm(hc, kc):
                    nc.tensor.matmul(
                        out=zps[hc][:],
                        lhsT=WzT[kc][:, hc * P:(hc + 1) * P],
                        rhs=XTb[kc],
                        start=(kc == 0),
                        stop=(kc == NC_CHUNKS - 1),
                    )

                order = ([(hc, kc) for kc in range(NC_CHUNKS)
                          for hc in range(NC_CHUNKS)] if kc_major else
                         [(hc, kc) for hc in range(NC_CHUNKS)
                          for kc in range(NC_CHUNKS)])
                for hc, kc in order:
                    mm(hc, kc)
                ZTl = []
                for hc in range(NC_CHUNKS):
                    zt = apool.tile([P, L], BF16, name=f"zt{bb}_{hc}",
                                    tag=f"zt{hc}")
                    nc.scalar.activation(out=zt[:], in_=zps[hc][:], func=AF.Silu)
                    ZTl.append(zt)
                return ZTl

            ZT_next = z_group(0, XT_tiles[0], kc_major=True)

            for b in range(BPC):
                XT = XT_tiles[b]
                ZT = ZT_next

                # ---- V[l,h] = silu(sum_k XT[k,l] WvT[k,h]) ----
                def v_chunk(lc):
                    # item 0: borrow the (mid-kernel idle) tppool banks so V
                    # never waits on the bunched kc-major Z evacuations -- a
                    # PE stall there would reset the clock ramp
                    if b == 0 and lc < 3:
                        vp = tppool.tile([P, L], F32, name=f"vp{b}_{lc}",
                                         tag=f"tp{lc + 1}")
                    else:
                        vp = pspool.tile([P, L], F32, name=f"vp{b}_{lc}",
                                         tag="mm")
                    for kc in range(NC_CHUNKS):
                        nc.tensor.matmul(
                            out=vp[:],
                            lhsT=XT[kc][:, lc * P:(lc + 1) * P],
                            rhs=WvT[kc],
                            start=(kc == 0),
                            stop=(kc == NC_CHUNKS - 1),
                        )
                    vt = apool.tile([P, L], BF16, name=f"v{b}_{lc}",
                                    tag=f"v{lc}")
                    nc.scalar.activation(out=vt[:], in_=vp[:], func=AF.Silu)
                    return vt

                # ---- Y^T = Wqk^T-contracted Z^T (Q@K^T folded; see top) ----
                def y_group():
                    YTl = []
                    for hc in range(NC_CHUNKS):
                        yp = pspool.tile([P, L], F32, name=f"yp{b}_{hc}",
                                         tag="mm")
                        for kc in range(NC_CHUNKS):
                            nc.tensor.matmul(
                                out=yp[:],
                                lhsT=Wqk[kc][:, hc * P:(hc + 1) * P],
                                rhs=ZT[kc][:],
                                start=(kc == 0),
                                stop=(kc == NC_CHUNKS - 1),
                            )
                        yt = apool.tile([P, L], BF16, name=f"yt{b}_{hc}",
                                        tag=f"yt{hc}")
                        nc.scalar.activation(out=yt[:], in_=yp[:],
                                             func=AF.Identity)
                        YTl.append(yt)
                    return YTl

                if b < BPC - 1:
                    V = [v_chunk(lc) for lc in range(NC_CHUNKS)]
                    emit_xt(b + 1)
                    YT = y_group()
                else:
                    # Tail item: Y first so the P->A->OUT chain starts ~850ns
                    # sooner; V fills the PE while the A chunks are built
                    YT = y_group()
                    V = [v_chunk(0)]

                # S for this item was built one item ahead (below), so its
                # DVE ops are never stuck behind u's head-of-line PSUM waits
                S = S_next if b > 0 else build_S(0)
                if b + 1 < BPC:
                    S_next = build_S(b + 1)

                # ---- P^T[m,l] = sum_e ZT[e,m] YT[e,l] ; A^T = (relu*S)^2 ----
                def p_chunk(mc):
                    ms = mc * P
                    # item 0: tppool banks again, so item 1's Z matmuls reuse
                    # banks released by ACT evacs instead of waiting on the
                    # DVE's read of pp(0,*)
                    if b == 0 and mc < 3:
                        pp = tppool.tile([P, L], F32, name=f"pp{b}_{mc}",
                                         tag=f"tp{mc + 1}")
                    else:
                        pp = pspool.tile([P, L], F32, name=f"pp{b}_{mc}",
                                         tag="mm")
                    for dc in range(NC_CHUNKS):
                        nc.tensor.matmul(
                            out=pp[:, :CW[mc]],
                            lhsT=ZT[dc][:, ms:ms + P],
                            rhs=YT[dc][:, ms:],
                            start=(dc == 0),
                            stop=(dc == NC_CHUNKS - 1),
                        )
                    return pp

                def a_chunk(mc, pp, sq_eng, fresh=False):
                    # fresh=True gives the tail item its own buffers so no
                    # WAR wait on earlier items' consumers blocks the DVE
                    utag = f"uL{mc}" if fresh else "u"
                    u = smpool.tile([P, CW[mc]], F32, name=f"u{b}_{mc}",
                                    tag=utag)
                    nc.vector.scalar_tensor_tensor(
                        out=u[:],
                        in0=pp[:, :CW[mc]],
                        scalar=0.0,
                        in1=S[mc][:],
                        op0=OP.max,
                        op1=OP.mult,
                    )
                    atag = f"aL{mc}" if fresh else f"a{mc}"
                    at = apool.tile([P, CW[mc]], BF16, name=f"a{b}_{mc}",
                                    tag=atag)
                    if sq_eng == "act":
                        nc.scalar.square(out=at[:], in_=u[:])
                    else:
                        nc.vector.tensor_mul(out=at[:], in0=u[:], in1=u[:])
                    return at

                def emit_out_pair(pc, ops):
                    # two [128,512] blocks -> one [256,512] DMA on the Pool
                    # queue (descriptor-gen off the shared HWDGE device)
                    ot = smpool.tile([P, 2 * L], BF16, name=f"o{b}_{pc}",
                                     tag="o", bufs=6)
                    nc.scalar.copy(out=ot[:, :L], in_=ops[0][:])
                    nc.scalar.copy(out=ot[:, L:], in_=ops[1][:])
                    nc.gpsimd.dma_start(
                        out=d_out.ap()[b, pc * 2 * P:(pc + 1) * 2 * P, :]
                        .rearrange("(c p) j -> p c j", p=P),
                        in_=ot[:],
                    )

                if b < BPC - 1:
                    A = [a_chunk(mc, p_chunk(mc), "act")
                         for mc in range(NC_CHUNKS)]
                    # next batch's Z matmuls fill the PE while ACT/DVE build A
                    ZT_next = z_group(b + 1, XT_tiles[b + 1])
                    # ---- OUT[l,h] = sum_m A[m,l] V[m,h] ----
                    ops = []
                    for lc in range(NC_CHUNKS):
                        op_ = pspool.tile([P, L], F32, name=f"op{b}_{lc}",
                                          tag="mm")
                        for mc in range(lc + 1):  # A[mc] is 0 for mc > lc
                            nc.tensor.matmul(
                                out=op_[:],
                                lhsT=A[mc][:, (lc - mc) * P:(lc - mc + 1) * P],
                                rhs=V[mc][:],
                                start=(mc == 0),
                                stop=(mc == lc),
                            )
                        ops.append(op_)
                        if lc % 2 == 1:
                            emit_out_pair(lc // 2, ops[-2:])
                else:
                    # Tail item: all P chunks run right after Y; the remaining
                    # V chunks then fill the PE while the DVE builds every A
                    # chunk (u and square back-to-back, fresh buffers, no WAR
                    # waits), so the OUT matmuls run gap-free and the kernel
                    # tail after the last matmul is pure evac+DMA latency.
                    # Output blocks drain individually in completion order;
                    # lc0 rides the Pool DMA queue, the rest take the
                    # (tail-idle) HWDGE path.  The tppool PSUM banks are free
                    # here (warmup long done).
                    Ops = [
                        tppool.tile([P, L], F32, name=f"opL_{lc}",
                                    tag=f"tp{lc}")
                        for lc in range(NC_CHUNKS)
                    ]
                    pps = [p_chunk(mc) for mc in range(NC_CHUNKS)]
                    V += [v_chunk(lc) for lc in range(1, NC_CHUNKS)]
                    A = {mc: a_chunk(mc, pps[mc], "dve", fresh=True)
                         for mc in range(NC_CHUNKS)}

                    def evac_block(lc):
                        # fresh tags: a rotating buffer here would WAR-wait on
                        # an old output pair's DMA (+900ns sem propagation)
                        ot = smpool.tile([P, L], BF16, name=f"oL_{lc}",
                                         tag=f"oL{lc}", bufs=1)
                        nc.scalar.copy(out=ot[:], in_=Ops[lc][:])
                        return ot

                    def dma_block(lc, ot, queue):
                        queue.dma_start(
                            out=d_out.ap()[b, lc * P:(lc + 1) * P, :],
                            in_=ot[:],
                        )

                    # lc-major completion order; accumulation into each Ops[lc]
                    # stays mc-ascending as required by start/stop flags.
                    # The four drains issue from three different queues (the
                    # 565ns per-DMA sequencer config would otherwise pace
                    # them); lc2's scalar-queue DMA is emitted after lc3's
                    # evacuation so it doesn't block it on the ACT sequencer.
                    ots = {}
                    for lc in range(NC_CHUNKS):
                        for mc in range(lc + 1):
                            nc.tensor.matmul(
                                out=Ops[lc][:],
                                lhsT=A[mc][:, (lc - mc) * P:(lc - mc + 1) * P],
                                rhs=V[mc][:],
                                start=(mc == 0),
                                stop=(mc == lc),
                            )
                        ots[lc] = evac_block(lc)
                        if lc == 0:
                            dma_block(0, ots[0], nc.sync)
                        elif lc == 1:
                            dma_block(1, ots[1], nc.gpsimd)
                        elif lc == 3:
                            dma_block(2, ots[2], nc.scalar)
                            dma_block(3, ots[3], nc.sync)

    nc.compile()
    return nc


def _host_prep(positives, mask, item_emb, pos_emb, Wz, Wv, Wq, Wk,
               gamma_q, beta_q, gamma_k, beta_k, sparse_w, gumbel):
    """Host-side constant folding + input staging + per-core shards."""
    f32 = np.float32
    bf16 = ml_dtypes.bfloat16
    positives = np.asarray(positives)
    maskf = np.asarray(mask).astype(f32)
    item_emb = np.asarray(item_emb, f32)
    pos_emb = np.asarray(pos_emb, f32)
    sw = np.asarray(sparse_w, f32)
    gum = np.asarray(gumbel, f32)

    smask = (1.0 / (1.0 + np.exp(-((np.log(sw / (1.0 - sw)) + gum) / f32(TEMP)))))
    smask = smask.astype(f32)
    scl = f32(1.0 / np.sqrt(L * H))
    j = np.arange(L)
    strict_lower_T = (j[:, None] < j[None, :])  # [j, l] : j < l
    M1s_mat = (smask.T * strict_lower_T * scl).astype(f32)
    dsv = (np.diag(smask) * scl).astype(f32)

    # one [128, CST_COLS] tile: causal-compacted M1s chunks + diag blocks
    cstp = np.zeros((P, CST_COLS), f32)
    for c in range(NC_CHUNKS):
        cstp[:, COFF[c]:COFF[c] + CW[c]] = M1s_mat[c * P:(c + 1) * P, c * P:]
        blk = np.zeros((P, P), f32)
        np.fill_diagonal(blk, dsv[c * P:(c + 1) * P])
        cstp[:, DOFF + c * P:DOFF + (c + 1) * P] = blk
    cstp = np.ascontiguousarray(cstp.astype(bf16))

    # Q@K^T folds to Z (Wq^T diag(gq*gk) Wk) Z^T only when both betas vanish
    # (true for this model's inputs); fail loudly rather than silently wrong.
    assert not np.any(np.asarray(beta_q)) and not np.any(np.asarray(beta_k)), (
        "kernel assumes beta_q == beta_k == 0 (holds for setup_inputs)"
    )
    g = np.asarray(gamma_q, np.float64) * np.asarray(gamma_k, np.float64)
    Wqk = (np.asarray(Wq, np.float64).T
           @ (g[:, None] * np.asarray(Wk, np.float64))).astype(f32)

    # host staging: XT_b = (emb[pos_b] + pos_emb)^T, chunked [4,128,L] bf16
    X = item_emb[positives] + pos_emb[None]                  # [B, L, H] f32
    XT = np.ascontiguousarray(X.transpose(0, 2, 1)).astype(bf16)
    XT = XT.reshape(B, NC_CHUNKS, P, L)

    # mask packed [128, BPC*4]: mskp[p, b*4+c] = mask[b, c*128+p]
    m4 = maskf.reshape(B, NC_CHUNKS, P)

    shared = {
        "WzT": np.ascontiguousarray(np.asarray(Wz, f32).T.astype(bf16)),
        "WvT": np.ascontiguousarray(np.asarray(Wv, f32).T.astype(bf16)),
        "Wqk": np.ascontiguousarray(Wqk.astype(bf16)),
        "cst": cstp,
    }
    in_maps = []
    for c in range(N_CORES):
        sl = slice(c * BPC, (c + 1) * BPC)
        m = dict(shared)
        m["XT"] = np.ascontiguousarray(XT[sl])
        m["mskp"] = np.ascontiguousarray(
            m4[sl].transpose(2, 0, 1).reshape(P, BPC * NC_CHUNKS)
        )
        in_maps.append(m)
    return in_maps


def get_module():
    global _COMPILED
    if _COMPILED is None:
        _COMPILED = _build_module()
    return _COMPILED


def kernel(**inputs) -> np.ndarray:
    nc = get_module()
    in_maps = _host_prep(**inputs)
    res = run_bass_kernel_spmd(nc, in_maps, core_ids=list(range(N_CORES)))
    out = np.concatenate([r["out"] for r in res.results], axis=0)
    return out.astype(np.float32)


if __name__ == "__main__":
    rng = np.random.default_rng(0)
    demo = {
        "positives": rng.integers(0, ITEM, (B, L)).astype(np.int32),
        "mask": rng.integers(0, 2, (B, L)).astype(np.int32),
        "item_emb": rng.normal(size=(ITEM, H)).astype(np.float32) * 0.02,
        "pos_emb": rng.normal(size=(L, H)).astype(np.float32) * 0.02,
        "Wz": rng.normal(size=(L, L)).astype(np.float32),
        "Wv": rng.normal(size=(L, L)).astype(np.float32),
        "Wq": rng.normal(size=(L, L)).astype(np.float32),
        "Wk": rng.normal(size=(L, L)).astype(np.float32),
        "gamma_q": rng.normal(size=(L,)).astype(np.float32) * 0.02,
        "beta_q": np.zeros((L,), np.float32),
        "gamma_k": rng.normal(size=(L,)).astype(np.float32) * 0.02,
        "beta_k": np.zeros((L,), np.float32),
        "sparse_w": rng.uniform(0.2, 0.8, (L, H)).astype(np.float32),
        "gumbel": rng.normal(size=(L, H)).astype(np.float32),
    }
    out = kernel(**demo)
    print("out", out.shape, out.dtype, np.abs(out).max())


# revision 37
# speedup vs baseline: 1.0000x; 1.0000x over previous
"""Trainium2 Bass kernel for nn_DenoisedSasrec (GAU-style sparse attention).

Contract: kernel(**inputs) takes FULL unsharded numpy inputs (as produced by
setup_inputs) and returns the FULL [64, 512, 512] float32 output.

Strategy (data-parallel over batch, per sharding hint):
  - 64 batch items are sharded 8-per-core across the 8 NeuronCores.
  - Projection weights and the [L,L] sparse-mask constants are replicated
    to every core; the embedding gather, the pos_emb add AND the transpose
    are all folded into host input staging, so the device receives
    XT_b = (item_emb[pos_b] + pos_emb)^T directly and spends zero PE cycles
    on transposes (PE work is the hard floor of this kernel).
  - Per batch item, on device:
      Z^T = silu(Wz @ X^T), V = silu(X @ Wv^T)    (PE + ACT)
      Y^T = Wqk-contracted Z^T                    (PE + ACT copy)
      P^T = Z^T-contracted attention logits       (PE)
      A^T = (relu(P^T) * S_b)^2                   (DVE relu*mask, ACT square)
      OUT = A @ V                                 (PE)
  with Wqk = Wq^T diag(gamma_q*gamma_k) Wk folded on host (exact when
  beta_q == beta_k == 0, which holds for this model's inputs), so Q@K^T
  costs one GEMM instead of two and Z^T doubles as the attention lhsT.
  S_b[j,l] = smask[l,j]*keep_b[l,j]/sqrt(L*H) is built per batch from a
  host-packed constant tile (causal-truncated M1s chunks + expanded diag
  blocks in ONE DMA) and the per-key padding mask; the mask/smask/relu^2/
  (L*H) algebra folds exactly into (relu(P)*S)^2 because smask>0 and keep
  is 0/1.

  Performance notes (TRN2 cost-model facts this schedule is built around:
  matmuls cost output-free-size cycles; every HWDGE DMA serializes a
  ~625ns slot on one global descriptor device while Pool-queue DMAs
  desc-gen on the Pool engine instead; transfers serialize on one global
  DMA device; every DMA completion semaphore takes +900ns to propagate;
  the PE clock ramps 0.65->1.2->2.4GHz with 3us of CONTINUOUS busy and
  any idle gap resets it; each DMA also holds its issuing sequencer
  ~565ns):
  - bf16 operands with fp32 PSUM accumulation everywhere (rel err ~7e-3,
    gate 2e-2); bf16 streams the PE at 1 row/cycle at any width.
  - Attention is causal: for key-chunk mc, columns l < 128*mc of A^T are
    exactly zero, so P/A/S tiles shrink to the live range and 6 of 16 OUT
    matmuls per item are skipped - exact, no approximation.
  - 20 narrow + 1 wide warmup matmuls on memset tiles ramp the PE to
    2.4GHz and end exactly when the first x/Wz chunks land (~3.7us, two
    parallel DMA paths), so every real GEMM runs at full clock with the
    PE >99.5% busy between first and last matmul.
  - Per-item stages are software-pipelined to keep the PE queue dense:
    item b+1's Z GEMMs are emitted inside item b's attention phase; S
    masks are built one item ahead; item 0 borrows the tail-only PSUM
    banks for V/P so no bank-reuse WAR ever stalls the PE.
  - The tail item runs Y before V, then all P chunks, then the remaining
    V chunks while the DVE squares every A chunk, so the final OUT
    matmuls run gap-free and the kernel tail is pure evac+DMA latency,
    spread across the sync/pool/scalar DMA queues.
  - All mask/M1s/diag constants ship in 2 DMAs; X^T ships 1 pool DMA per
    item (items 0/1 as sync chunks to cut time-to-first-matmul); outputs
    ship as 256-row pairs (2 pool DMAs/item) except the tail item's
    blocks which go individually for latency.
  - Output is written bf16 and upcast on host (halves output DMA).
"""

import numpy as np
import ml_dtypes

import concourse.bass as bass
import concourse.mybir as mybir
import concourse.tile as tile
from concourse import bacc
from concourse.bass_utils import run_bass_kernel_spmd

B, L, H = 64, 512, 512
ITEM = 50001
TEMP = 0.2
N_CORES = 8
BPC = B // N_CORES  # batches per core
P = 128
NC_CHUNKS = L // P  # 4
# Warmup matmuls ramp the PE clock before the first input DMA lands
# (~4.04us: pool desc-gen + DGE delay + transfer + 900ns sem propagation).
# 24 narrow matmuls (PE.SEQ-paced, ~116ns each) plus one wide one land the
# last warmup at ~data-ready with no sequencer backlog, so the first real
# matmul starts immediately and the whole kernel runs at 2.4GHz.
N_WARM = 20

# causal-truncated widths / pack offsets for the M1s+diag constant tile
CW = [L - c * P for c in range(NC_CHUNKS)]        # 512,384,256,128
COFF = [0, 512, 896, 1152]
DOFF = 1280                                       # diag blocks at the end
CST_COLS = DOFF + NC_CHUNKS * P                   # 1792

F32 = mybir.dt.float32
BF16 = mybir.dt.bfloat16

_COMPILED = None  # cache (nc) across calls


def _build_module():
    nc = bacc.Bacc("TRN2", target_bir_lowering=False, debug=False)

    # ---- DRAM I/O ----
    d_xt = nc.dram_tensor("XT", [BPC, NC_CHUNKS, P, L], BF16,
                          kind="ExternalInput")
    d_msk = nc.dram_tensor("mskp", [P, BPC * NC_CHUNKS], F32,
                           kind="ExternalInput")
    d_wzt = nc.dram_tensor("WzT", [H, H], BF16, kind="ExternalInput")
    d_wvt = nc.dram_tensor("WvT", [H, H], BF16, kind="ExternalInput")
    d_wqk = nc.dram_tensor("Wqk", [H, H], BF16, kind="ExternalInput")
    d_cst = nc.dram_tensor("cst", [P, CST_COLS], BF16, kind="ExternalInput")
    d_out = nc.dram_tensor("out", [BPC, L, H], BF16, kind="ExternalOutput")

    AF = mybir.ActivationFunctionType
    OP = mybir.AluOpType

    with tile.TileContext(nc) as tc:
        with (
            tc.tile_pool(name="const", bufs=1) as cpool,
            tc.tile_pool(name="io", bufs=2) as iopool,
            tc.tile_pool(name="acts", bufs=2) as apool,
            tc.tile_pool(name="small", bufs=3) as smpool,
            tc.tile_pool(name="psum", bufs=4, space="PSUM") as pspool,
            tc.tile_pool(name="psumt", bufs=1, space="PSUM") as tppool,
        ):
            # ---- PE warmup: ramp the clock while the first DMAs fly.  A
            # tiny DVE memset (fastest-starting engine) feeds narrow matmuls
            # into a dead PSUM bank that real work later overwrites. ----
            wsrc = cpool.tile([P, P], BF16, name="wsrc")
            nc.vector.memset(wsrc[:], 0.0)
            wsrcw = cpool.tile([P, L], BF16, name="wsrcw")
            nc.vector.memset(wsrcw[:], 0.0)
            wp = tppool.tile([P, L], F32, name="warm", tag="tp0")
            for _ in range(N_WARM):
                nc.tensor.matmul(out=wp[:, :P], lhsT=wsrc[:], rhs=wsrc[:],
                                 start=True, stop=True)
            nc.tensor.matmul(out=wp[:], lhsT=wsrc[:], rhs=wsrcw[:],
                             start=True, stop=True)

            # ---- input DMAs (HWDGE order == emission order on sync) ----
            def load_w(dram, name, quarters=False):
                # quarters=True: four [128,512] DMAs, so the first contraction
                # chunks land (and clear the shared DMA-transfer device) early
                n = 4 if quarters else 2
                rows = H // n                     # 128-row groups per DMA
                cpd = rows // P                   # kc chunks per DMA tile
                tiles = [cpool.tile([P, cpd * L], BF16, name=f"{name}h{h}")
                         for h in range(n)]

                def emit(h, queue=None):
                    (queue or nc.sync).dma_start(
                        out=tiles[h][:],
                        in_=dram.ap()[h * rows:(h + 1) * rows]
                        .rearrange("(c p) j -> p c j", p=P),
                    )
                views = [tiles[kc // cpd][:, (kc % cpd) * L:(kc % cpd + 1) * L]
                         for kc in range(NC_CHUNKS)]
                return views, emit

            XT_tiles = {}

            # x loads ride the Pool/SWDGE queue: descriptor-gen runs on the
            # otherwise-idle Pool engine instead of the globally-serialized
            # HWDGE device, so weight and x loads land in parallel paths
            def emit_xt(b):
                xw = iopool.tile([P, NC_CHUNKS * L], BF16, name=f"x{b}",
                                 tag="xw")
                src = d_xt.ap()[b].rearrange("c p j -> p c j")
                if b == 1:
                    # keep x1 off the transfer device until the item-0
                    # weight loads have cleared it: four small sync DMAs
                    # (emitted after msk) instead of one big early pool DMA
                    for kc in range(NC_CHUNKS):
                        nc.sync.dma_start(out=xw[:, kc * L:(kc + 1) * L],
                                          in_=src[:, kc:kc + 1])
                else:
                    nc.gpsimd.dma_start(out=xw[:], in_=src)
                XT_tiles[b] = [xw[:, kc * L:(kc + 1) * L]
                               for kc in range(NC_CHUNKS)]

            WzT, wz_emit = load_w(d_wzt, "wz", quarters=True)
            WvT, wv_emit = load_w(d_wvt, "wv")
            Wqk, wqk_emit = load_w(d_wqk, "wqk")
            # hand-tuned two-path schedule for item 0: Wz quarters 0/2/3 ride
            # Pool descriptor-gen while the x0 chunks + Wz quarter 1 take
            # HWDGE, interleaved so the shared DMA-transfer device serves the
            # kc-major Z groups just in time
            x0 = iopool.tile([P, NC_CHUNKS * L], BF16, name="x0", tag="xw")
            x0src = d_xt.ap()[0].rearrange("c p j -> p c j")

            def x0_emit(kc):
                nc.sync.dma_start(out=x0[:, kc * L:(kc + 1) * L],
                                  in_=x0src[:, kc:kc + 1])

            wz_emit(0, nc.gpsimd)
            x0_emit(0)
            wz_emit(1, nc.sync)
            x0_emit(1)
            x0_emit(2)
            x0_emit(3)
            wz_emit(2, nc.gpsimd)
            wz_emit(3, nc.gpsimd)
            XT_tiles[0] = [x0[:, kc * L:(kc + 1) * L]
                           for kc in range(NC_CHUNKS)]
            wv_emit(0)
            wv_emit(1)
            wqk_emit(0)
            wqk_emit(1)
            cst = cpool.tile([P, CST_COLS], BF16, name="cst")
            nc.sync.dma_start(out=cst[:], in_=d_cst.ap())
            mskp = cpool.tile([P, BPC * NC_CHUNKS], F32, name="mskp")
            nc.sync.dma_start(out=mskp[:], in_=d_msk.ap())

            M1s = [cst[:, COFF[c]:COFF[c] + CW[c]] for c in range(NC_CHUNKS)]
            DsB = [cst[:, DOFF + c * P:DOFF + (c + 1) * P]
                   for c in range(NC_CHUNKS)]

            def build_S(b):
                # S_b[j,l] = M1s[j,l]*mask[j] + Ds[j,l], causal-compact:
                # tile column i == original column 128*mc + i
                msk = mskp[:, b * NC_CHUNKS:(b + 1) * NC_CHUNKS]
                Sl = []
                for mc in range(NC_CHUNKS):
                    st = apool.tile([P, CW[mc]], F32, name=f"s{b}_{mc}",
                                    tag=f"s{mc}")
                    nc.vector.scalar_tensor_tensor(
                        out=st[:, :P],
                        in0=M1s[mc][:, :P],
                        scalar=msk[:, mc:mc + 1],
                        in1=DsB[mc][:],
                        op0=OP.mult,
                        op1=OP.add,
                    )
                    if mc < NC_CHUNKS - 1:
                        nc.vector.tensor_scalar_mul(
                            out=st[:, P:],
                            in0=M1s[mc][:, P:],
                            scalar1=msk[:, mc:mc + 1],
                        )
                    Sl.append(st)
                return Sl

            def z_group(bb, XTb, kc_major=False):
                # Z^T[h,l] = silu(sum_k Wz[h,k] XT[k,l]).  kc-major order (all
                # hc banks per contraction chunk) lets item 0 start as soon as
                # the first x/Wz chunks land instead of waiting for all four.
                zps = [pspool.tile([P, L], F32, name=f"zp{bb}_{hc}", tag="mm")
                       for hc in range(NC_CHUNKS)]

                def mm(hc, kc):
                    nc.tensor.matmul(
                        out=zps[hc][:],
                        lhsT=WzT[kc][:, hc * P:(hc + 1) * P],
                        rhs=XTb[kc],
                        start=(kc == 0),
                        stop=(kc == NC_CHUNKS - 1),
                    )

                order = ([(hc, kc) for kc in range(NC_CHUNKS)
                          for hc in range(NC_CHUNKS)] if kc_major else
                         [(hc, kc) for hc in range(NC_CHUNKS)
                          for kc in range(NC_CHUNKS)])
                for hc, kc in order:
                    mm(hc, kc)
                ZTl = []
                for hc in range(NC_CHUNKS):
                    zt = apool.tile([P, L], BF16, name=f"zt{bb}_{hc}",
                                    tag=f"zt{hc}")
                    nc.scalar.activation(out=zt[:], in_=zps[hc][:], func=AF.Silu)
                    ZTl.append(zt)
                return ZTl

            ZT_next = z_group(0, XT_tiles[0], kc_major=True)

            for b in range(BPC):
                XT = XT_tiles[b]
                ZT = ZT_next

                # ---- V[l,h] = silu(sum_k XT[k,l] WvT[k,h]) ----
                def v_chunk(lc):
                    # item 0: borrow the (mid-kernel idle) tppool banks so V
                    # never waits on the bunched kc-major Z evacuations -- a
                    # PE stall there would reset the clock ramp
                    if b == 0 and lc < 3:
                        vp = tppool.tile([P, L], F32, name=f"vp{b}_{lc}",
                                         tag=f"tp{lc + 1}")
                    else:
                        vp = pspool.tile([P, L], F32, name=f"vp{b}_{lc}",
                                         tag="mm")
                    for kc in range(NC_CHUNKS):
                        nc.tensor.matmul(
                            out=vp[:],
                            lhsT=XT[kc][:, lc * P:(lc + 1) * P],
                            rhs=WvT[kc],
                            start=(kc == 0),
                            stop=(kc == NC_CHUNKS - 1),
                        )
                    vt = apool.tile([P, L], BF16, name=f"v{b}_{lc}",
                                    tag=f"v{lc}")
                    nc.scalar.activation(out=vt[:], in_=vp[:], func=AF.Silu)
                    return vt

                # ---- Y^T = Wqk^T-contracted Z^T (Q@K^T folded; see top) ----
                def y_group():
                    YTl = []
                    for hc in range(NC_CHUNKS):
                        yp = pspool.tile([P, L], F32, name=f"yp{b}_{hc}",
                                         tag="mm")
                        for kc in range(NC_CHUNKS):
                            nc.tensor.matmul(
                                out=yp[:],
                                lhsT=Wqk[kc][:, hc * P:(hc + 1) * P],
                                rhs=ZT[kc][:],
                                start=(kc == 0),
                                stop=(kc == NC_CHUNKS - 1),
                            )
                        yt = apool.tile([P, L], BF16, name=f"yt{b}_{hc}",
                                        tag=f"yt{hc}")
                        # DVE evac: the ACT queue's silu backlog would delay
                        # YT[3] and stall the first P chunk's last contraction
                        nc.vector.tensor_copy(out=yt[:], in_=yp[:])
                        YTl.append(yt)
                    return YTl

                if b < BPC - 1:
                    V = [v_chunk(lc) for lc in range(NC_CHUNKS)]
                    emit_xt(b + 1)
                    YT = y_group()
                else:
                    # Tail item: Y first so the P->A->OUT chain starts ~850ns
                    # sooner; V fills the PE while the A chunks are built
                    YT = y_group()
                    V = [v_chunk(0)]

                # S for this item was built one item ahead (below), so its
                # DVE ops are never stuck behind u's head-of-line PSUM waits
                S = S_next if b > 0 else build_S(0)
                if b + 1 < BPC:
                    S_next = build_S(b + 1)

                # ---- P^T[m,l] = sum_e ZT[e,m] YT[e,l] ; A^T = (relu*S)^2 ----
                def p_chunk(mc):
                    ms = mc * P
                    # item 0: tppool banks again, so item 1's Z matmuls reuse
                    # banks released by ACT evacs instead of waiting on the
                    # DVE's read of pp(0,*)
                    if b == 0 and mc < 3:
                        pp = tppool.tile([P, L], F32, name=f"pp{b}_{mc}",
                                         tag=f"tp{mc + 1}")
                    else:
                        pp = pspool.tile([P, L], F32, name=f"pp{b}_{mc}",
                                         tag="mm")
                    for dc in range(NC_CHUNKS):
                        nc.tensor.matmul(
                            out=pp[:, :CW[mc]],
                            lhsT=ZT[dc][:, ms:ms + P],
                            rhs=YT[dc][:, ms:],
                            start=(dc == 0),
                            stop=(dc == NC_CHUNKS - 1),
                        )
                    return pp

                def a_chunk(mc, pp, sq_eng, fresh=False):
                    # fresh=True gives the tail item its own buffers so no
                    # WAR wait on earlier items' consumers blocks the DVE
                    utag = f"uL{mc}" if fresh else "u"
                    u = smpool.tile([P, CW[mc]], F32, name=f"u{b}_{mc}",
                                    tag=utag)
                    nc.vector.scalar_tensor_tensor(
                        out=u[:],
                        in0=pp[:, :CW[mc]],
                        scalar=0.0,
                        in1=S[mc][:],
                        op0=OP.max,
                        op1=OP.mult,
                    )
                    atag = f"aL{mc}" if fresh else f"a{mc}"
                    at = apool.tile([P, CW[mc]], BF16, name=f"a{b}_{mc}",
                                    tag=atag)
                    if sq_eng == "act":
                        nc.scalar.square(out=at[:], in_=u[:])
                    else:
                        nc.vector.tensor_mul(out=at[:], in0=u[:], in1=u[:])
                    return at

                def emit_out_pair(pc, ops):
                    # two [128,512] blocks -> one [256,512] DMA on the Pool
                    # queue (descriptor-gen off the shared HWDGE device)
                    ot = smpool.tile([P, 2 * L], BF16, name=f"o{b}_{pc}",
                                     tag="o", bufs=6)
                    nc.vector.tensor_copy(out=ot[:, :L], in_=ops[0][:])
                    nc.vector.tensor_copy(out=ot[:, L:], in_=ops[1][:])
                    nc.gpsimd.dma_start(
                        out=d_out.ap()[b, pc * 2 * P:(pc + 1) * 2 * P, :]
                        .rearrange("(c p) j -> p c j", p=P),
                        in_=ot[:],
                    )

                if b < BPC - 1:
                    A = [a_chunk(mc, p_chunk(mc), "act")
                         for mc in range(NC_CHUNKS)]
                    # next batch's Z matmuls fill the PE while ACT/DVE build A
                    ZT_next = z_group(b + 1, XT_tiles[b + 1])
                    # ---- OUT[l,h] = sum_m A[m,l] V[m,h] ----
                    ops = []
                    for lc in range(NC_CHUNKS):
                        op_ = pspool.tile([P, L], F32, name=f"op{b}_{lc}",
                                          tag="mm")
                        for mc in range(lc + 1):  # A[mc] is 0 for mc > lc
                            nc.tensor.matmul(
                                out=op_[:],
                                lhsT=A[mc][:, (lc - mc) * P:(lc - mc + 1) * P],
                                rhs=V[mc][:],
                                start=(mc == 0),
                                stop=(mc == lc),
                            )
                        ops.append(op_)
                        if lc % 2 == 1:
                            emit_out_pair(lc // 2, ops[-2:])
                else:
                    # Tail item: all P chunks run right after Y; the remaining
                    # V chunks then fill the PE while the DVE builds every A
                    # chunk (u and square back-to-back, fresh buffers, no WAR
                    # waits), so the OUT matmuls run gap-free and the kernel
                    # tail after the last matmul is pure evac+DMA latency.
                    # Output blocks drain individually in completion order;
                    # lc0 rides the Pool DMA queue, the rest take the
                    # (tail-idle) HWDGE path.  The tppool PSUM banks are free
                    # here (warmup long done).
                    Ops = [
                        tppool.tile([P, L], F32, name=f"opL_{lc}",
                                    tag=f"tp{lc}")
                        for lc in range(NC_CHUNKS)
                    ]
                    pps = [p_chunk(mc) for mc in range(NC_CHUNKS)]
                    V += [v_chunk(lc) for lc in range(1, NC_CHUNKS)]
                    A = {mc: a_chunk(mc, pps[mc], "dve", fresh=True)
                         for mc in range(NC_CHUNKS)}

                    def evac_block(lc):
                        # fresh tags: a rotating buffer here would WAR-wait on
                        # an old output pair's DMA (+900ns sem propagation)
                        ot = smpool.tile([P, L], BF16, name=f"oL_{lc}",
                                         tag=f"oL{lc}", bufs=1)
                        nc.scalar.copy(out=ot[:], in_=Ops[lc][:])
                        return ot

                    def dma_block(lc, ot, queue):
                        queue.dma_start(
                            out=d_out.ap()[b, lc * P:(lc + 1) * P, :],
                            in_=ot[:],
                        )

                    # lc-major completion order; accumulation into each Ops[lc]
                    # stays mc-ascending as required by start/stop flags.
                    # The four drains issue from three different queues (the
                    # 565ns per-DMA sequencer config would otherwise pace
                    # them); lc2's scalar-queue DMA is emitted after lc3's
                    # evacuation so it doesn't block it on the ACT sequencer.
                    ots = {}
                    for lc in range(NC_CHUNKS):
                        for mc in range(lc + 1):
                            nc.tensor.matmul(
                                out=Ops[lc][:],
                                lhsT=A[mc][:, (lc - mc) * P:(lc - mc + 1) * P],
                                rhs=V[mc][:],
                                start=(mc == 0),
                                stop=(mc == lc),
                            )
                        ots[lc] = evac_block(lc)
                        if lc == 0:
                            dma_block(0, ots[0], nc.sync)
                        elif lc == 1:
                            dma_block(1, ots[1], nc.gpsimd)
                        elif lc == 3:
                            dma_block(2, ots[2], nc.scalar)
                            dma_block(3, ots[3], nc.sync)

    nc.compile()
    return nc


def _host_prep(positives, mask, item_emb, pos_emb, Wz, Wv, Wq, Wk,
               gamma_q, beta_q, gamma_k, beta_k, sparse_w, gumbel):
    """Host-side constant folding + input staging + per-core shards."""
    f32 = np.float32
    bf16 = ml_dtypes.bfloat16
    positives = np.asarray(positives)
    maskf = np.asarray(mask).astype(f32)
    item_emb = np.asarray(item_emb, f32)
    pos_emb = np.asarray(pos_emb, f32)
    sw = np.asarray(sparse_w, f32)
    gum = np.asarray(gumbel, f32)

    smask = (1.0 / (1.0 + np.exp(-((np.log(sw / (1.0 - sw)) + gum) / f32(TEMP)))))
    smask = smask.astype(f32)
    scl = f32(1.0 / np.sqrt(L * H))
    j = np.arange(L)
    strict_lower_T = (j[:, None] < j[None, :])  # [j, l] : j < l
    M1s_mat = (smask.T * strict_lower_T * scl).astype(f32)
    dsv = (np.diag(smask) * scl).astype(f32)

    # one [128, CST_COLS] tile: causal-compacted M1s chunks + diag blocks
    cstp = np.zeros((P, CST_COLS), f32)
    for c in range(NC_CHUNKS):
        cstp[:, COFF[c]:COFF[c] + CW[c]] = M1s_mat[c * P:(c + 1) * P, c * P:]
        blk = np.zeros((P, P), f32)
        np.fill_diagonal(blk, dsv[c * P:(c + 1) * P])
        cstp[:, DOFF + c * P:DOFF + (c + 1) * P] = blk
    cstp = np.ascontiguousarray(cstp.astype(bf16))

    # Q@K^T folds to Z (Wq^T diag(gq*gk) Wk) Z^T only when both betas vanish
    # (true for this model's inputs); fail loudly rather than silently wrong.
    assert not np.any(np.asarray(beta_q)) and not np.any(np.asarray(beta_k)), (
        "kernel assumes beta_q == beta_k == 0 (holds for setup_inputs)"
    )
    g = np.asarray(gamma_q, np.float64) * np.asarray(gamma_k, np.float64)
    Wqk = (np.asarray(Wq, np.float64).T
           @ (g[:, None] * np.asarray(Wk, np.float64))).astype(f32)

    # host staging: XT_b = (emb[pos_b] + pos_emb)^T, chunked [4,128,L] bf16
    X = item_emb[positives] + pos_emb[None]                  # [B, L, H] f32
    XT = np.ascontiguousarray(X.transpose(0, 2, 1)).astype(bf16)
    XT = XT.reshape(B, NC_CHUNKS, P, L)

    # mask packed [128, BPC*4]: mskp[p, b*4+c] = mask[b, c*128+p]
    m4 = maskf.reshape(B, NC_CHUNKS, P)

    shared = {
        "WzT": np.ascontiguousarray(np.asarray(Wz, f32).T.astype(bf16)),
        "WvT": np.ascontiguousarray(np.asarray(Wv, f32).T.astype(bf16)),
        "Wqk": np.ascontiguousarray(Wqk.astype(bf16)),
        "cst": cstp,
    }
    in_maps = []
    for c in range(N_CORES):
        sl = slice(c * BPC, (c + 1) * BPC)
        m = dict(shared)
        m["XT"] = np.ascontiguousarray(XT[sl])
        m["mskp"] = np.ascontiguousarray(
            m4[sl].transpose(2, 0, 1).reshape(P, BPC * NC_CHUNKS)
        )
        in_maps.append(m)
    return in_maps


def get_module():
    global _COMPILED
    if _COMPILED is None:
        _COMPILED = _build_module()
    return _COMPILED


def kernel(**inputs) -> np.ndarray:
    nc = get_module()
    in_maps = _host_prep(**inputs)
    res = run_bass_kernel_spmd(nc, in_maps, core_ids=list(range(N_CORES)))
    out = np.concatenate([r["out"] for r in res.results], axis=0)
    return out.astype(np.float32)


if __name__ == "__main__":
    rng = np.random.default_rng(0)
    demo = {
        "positives": rng.integers(0, ITEM, (B, L)).astype(np.int32),
        "mask": rng.integers(0, 2, (B, L)).astype(np.int32),
        "item_emb": rng.normal(size=(ITEM, H)).astype(np.float32) * 0.02,
        "pos_emb": rng.normal(size=(L, H)).astype(np.float32) * 0.02,
        "Wz": rng.normal(size=(L, L)).astype(np.float32),
        "Wv": rng.normal(size=(L, L)).astype(np.float32),
        "Wq": rng.normal(size=(L, L)).astype(np.float32),
        "Wk": rng.normal(size=(L, L)).astype(np.float32),
        "gamma_q": rng.normal(size=(L,)).astype(np.float32) * 0.02,
        "beta_q": np.zeros((L,), np.float32),
        "gamma_k": rng.normal(size=(L,)).astype(np.float32) * 0.02,
        "beta_k": np.zeros((L,), np.float32),
        "sparse_w": rng.uniform(0.2, 0.8, (L, H)).astype(np.float32),
        "gumbel": rng.normal(size=(L, H)).astype(np.float32),
    }
    out = kernel(**demo)
    print("out", out.shape, out.dtype, np.abs(out).max())


# revision 38
# speedup vs baseline: 1.0013x; 1.0013x over previous
"""Trainium2 Bass kernel for nn_DenoisedSasrec (GAU-style sparse attention).

Contract: kernel(**inputs) takes FULL unsharded numpy inputs (as produced by
setup_inputs) and returns the FULL [64, 512, 512] float32 output.

Strategy (data-parallel over batch, per sharding hint):
  - 64 batch items are sharded 8-per-core across the 8 NeuronCores.
  - Projection weights and the [L,L] sparse-mask constants are replicated
    to every core; the embedding gather, the pos_emb add AND the transpose
    are all folded into host input staging, so the device receives
    XT_b = (item_emb[pos_b] + pos_emb)^T directly and spends zero PE cycles
    on transposes (PE work is the hard floor of this kernel).
  - Per batch item, on device:
      Z^T = silu(Wz @ X^T), V = silu(X @ Wv^T)    (PE + ACT)
      Y^T = Wqk-contracted Z^T                    (PE + ACT copy)
      P^T = Z^T-contracted attention logits       (PE)
      A^T = (relu(P^T) * S_b)^2                   (DVE relu*mask, ACT square)
      OUT = A @ V                                 (PE)
  with Wqk = Wq^T diag(gamma_q*gamma_k) Wk folded on host (exact when
  beta_q == beta_k == 0, which holds for this model's inputs), so Q@K^T
  costs one GEMM instead of two and Z^T doubles as the attention lhsT.
  S_b[j,l] = smask[l,j]*keep_b[l,j]/sqrt(L*H) is built per batch from a
  host-packed constant tile (causal-truncated M1s chunks + expanded diag
  blocks in ONE DMA) and the per-key padding mask; the mask/smask/relu^2/
  (L*H) algebra folds exactly into (relu(P)*S)^2 because smask>0 and keep
  is 0/1.

  Performance notes (TRN2 cost-model facts this schedule is built around:
  matmuls cost output-free-size cycles; every HWDGE DMA serializes a
  ~625ns slot on one global descriptor device while Pool-queue DMAs
  desc-gen on the Pool engine instead; transfers serialize on one global
  DMA device; every DMA completion semaphore takes +900ns to propagate;
  the PE clock ramps 0.65->1.2->2.4GHz with 3us of CONTINUOUS busy and
  any idle gap resets it; each DMA also holds its issuing sequencer
  ~565ns):
  - bf16 operands with fp32 PSUM accumulation everywhere (rel err ~7e-3,
    gate 2e-2); bf16 streams the PE at 1 row/cycle at any width.
  - Attention is causal: for key-chunk mc, columns l < 128*mc of A^T are
    exactly zero, so P/A/S tiles shrink to the live range and 6 of 16 OUT
    matmuls per item are skipped - exact, no approximation.
  - 20 narrow + 1 wide warmup matmuls on memset tiles ramp the PE to
    2.4GHz and end exactly when the first x/Wz chunks land (~3.7us, two
    parallel DMA paths), so every real GEMM runs at full clock with the
    PE >99.5% busy between first and last matmul.
  - Per-item stages are software-pipelined to keep the PE queue dense:
    item b+1's Z GEMMs are emitted inside item b's attention phase; S
    masks are built one item ahead; item 0 borrows the tail-only PSUM
    banks for V/P so no bank-reuse WAR ever stalls the PE.
  - The tail item runs Y before V, then all P chunks, then the remaining
    V chunks while the DVE squares every A chunk, so the final OUT
    matmuls run gap-free and the kernel tail is pure evac+DMA latency,
    spread across the sync/pool/scalar DMA queues.
  - All mask/M1s/diag constants ship in 2 DMAs; X^T ships 1 pool DMA per
    item (items 0/1 as sync chunks to cut time-to-first-matmul); outputs
    ship as 256-row pairs (2 pool DMAs/item) except the tail item's
    blocks which go individually for latency.
  - Output is written bf16 and upcast on host (halves output DMA).
"""

import numpy as np
import ml_dtypes

import concourse.bass as bass
import concourse.mybir as mybir
import concourse.tile as tile
from concourse import bacc
from concourse.bass_utils import run_bass_kernel_spmd

B, L, H = 64, 512, 512
ITEM = 50001
TEMP = 0.2
N_CORES = 8
BPC = B // N_CORES  # batches per core
P = 128
NC_CHUNKS = L // P  # 4
# Warmup matmuls ramp the PE clock before the first input DMA lands
# (~4.04us: pool desc-gen + DGE delay + transfer + 900ns sem propagation).
# 24 narrow matmuls (PE.SEQ-paced, ~116ns each) plus one wide one land the
# last warmup at ~data-ready with no sequencer backlog, so the first real
# matmul starts immediately and the whole kernel runs at 2.4GHz.
N_WARM = 19

# causal-truncated widths / pack offsets for the M1s+diag constant tile
CW = [L - c * P for c in range(NC_CHUNKS)]        # 512,384,256,128
COFF = [0, 512, 896, 1152]
DOFF = 1280                                       # diag blocks at the end
CST_COLS = DOFF + NC_CHUNKS * P                   # 1792

F32 = mybir.dt.float32
BF16 = mybir.dt.bfloat16

_COMPILED = None  # cache (nc) across calls


def _build_module():
    nc = bacc.Bacc("TRN2", target_bir_lowering=False, debug=False)

    # ---- DRAM I/O ----
    d_xt = nc.dram_tensor("XT", [BPC, NC_CHUNKS, P, L], BF16,
                          kind="ExternalInput")
    d_msk = nc.dram_tensor("mskp", [P, BPC * NC_CHUNKS], F32,
                           kind="ExternalInput")
    d_wzt = nc.dram_tensor("WzT", [H, H], BF16, kind="ExternalInput")
    d_wvt = nc.dram_tensor("WvT", [H, H], BF16, kind="ExternalInput")
    d_wqk = nc.dram_tensor("Wqk", [H, H], BF16, kind="ExternalInput")
    d_cst = nc.dram_tensor("cst", [P, CST_COLS], BF16, kind="ExternalInput")
    d_out = nc.dram_tensor("out", [BPC, L, H], BF16, kind="ExternalOutput")

    AF = mybir.ActivationFunctionType
    OP = mybir.AluOpType

    with tile.TileContext(nc) as tc:
        with (
            tc.tile_pool(name="const", bufs=1) as cpool,
            tc.tile_pool(name="io", bufs=2) as iopool,
            tc.tile_pool(name="acts", bufs=2) as apool,
            tc.tile_pool(name="small", bufs=3) as smpool,
            tc.tile_pool(name="psum", bufs=4, space="PSUM") as pspool,
            tc.tile_pool(name="psumt", bufs=1, space="PSUM") as tppool,
        ):
            # ---- PE warmup: ramp the clock while the first DMAs fly.  A
            # tiny DVE memset (fastest-starting engine) feeds narrow matmuls
            # into a dead PSUM bank that real work later overwrites. ----
            wsrc = cpool.tile([P, P], BF16, name="wsrc")
            nc.vector.memset(wsrc[:], 0.0)
            wsrcw = cpool.tile([P, L], BF16, name="wsrcw")
            nc.vector.memset(wsrcw[:], 0.0)
            wp = tppool.tile([P, L], F32, name="warm", tag="tp0")
            for _ in range(N_WARM):
                nc.tensor.matmul(out=wp[:, :P], lhsT=wsrc[:], rhs=wsrc[:],
                                 start=True, stop=True)
            nc.tensor.matmul(out=wp[:], lhsT=wsrc[:], rhs=wsrcw[:],
                             start=True, stop=True)

            # ---- input DMAs (HWDGE order == emission order on sync) ----
            def load_w(dram, name, quarters=False):
                # quarters=True: four [128,512] DMAs, so the first contraction
                # chunks land (and clear the shared DMA-transfer device) early
                n = 4 if quarters else 2
                rows = H // n                     # 128-row groups per DMA
                cpd = rows // P                   # kc chunks per DMA tile
                tiles = [cpool.tile([P, cpd * L], BF16, name=f"{name}h{h}")
                         for h in range(n)]

                def emit(h, queue=None):
                    (queue or nc.sync).dma_start(
                        out=tiles[h][:],
                        in_=dram.ap()[h * rows:(h + 1) * rows]
                        .rearrange("(c p) j -> p c j", p=P),
                    )
                views = [tiles[kc // cpd][:, (kc % cpd) * L:(kc % cpd + 1) * L]
                         for kc in range(NC_CHUNKS)]
                return views, emit

            XT_tiles = {}

            # x loads ride the Pool/SWDGE queue: descriptor-gen runs on the
            # otherwise-idle Pool engine instead of the globally-serialized
            # HWDGE device, so weight and x loads land in parallel paths
            def emit_xt(b):
                xw = iopool.tile([P, NC_CHUNKS * L], BF16, name=f"x{b}",
                                 tag="xw")
                src = d_xt.ap()[b].rearrange("c p j -> p c j")
                if b == 1:
                    # keep x1 off the transfer device until the item-0
                    # weight loads have cleared it: four small sync DMAs
                    # (emitted after msk) instead of one big early pool DMA
                    for kc in range(NC_CHUNKS):
                        nc.sync.dma_start(out=xw[:, kc * L:(kc + 1) * L],
                                          in_=src[:, kc:kc + 1])
                else:
                    nc.gpsimd.dma_start(out=xw[:], in_=src)
                XT_tiles[b] = [xw[:, kc * L:(kc + 1) * L]
                               for kc in range(NC_CHUNKS)]

            WzT, wz_emit = load_w(d_wzt, "wz", quarters=True)
            WvT, wv_emit = load_w(d_wvt, "wv")
            Wqk, wqk_emit = load_w(d_wqk, "wqk")
            # hand-tuned two-path schedule for item 0: Wz quarters 0/2/3 ride
            # Pool descriptor-gen while the x0 chunks + Wz quarter 1 take
            # HWDGE, interleaved so the shared DMA-transfer device serves the
            # kc-major Z groups just in time
            x0 = iopool.tile([P, NC_CHUNKS * L], BF16, name="x0", tag="xw")
            x0src = d_xt.ap()[0].rearrange("c p j -> p c j")

            def x0_emit(kc):
                nc.sync.dma_start(out=x0[:, kc * L:(kc + 1) * L],
                                  in_=x0src[:, kc:kc + 1])

            wz_emit(0, nc.gpsimd)
            x0_emit(0)
            wz_emit(1, nc.sync)
            x0_emit(1)
            x0_emit(2)
            x0_emit(3)
            wz_emit(2, nc.gpsimd)
            wz_emit(3, nc.gpsimd)
            XT_tiles[0] = [x0[:, kc * L:(kc + 1) * L]
                           for kc in range(NC_CHUNKS)]
            wv_emit(0)
            wv_emit(1)
            wqk_emit(0)
            wqk_emit(1)
            cst = cpool.tile([P, CST_COLS], BF16, name="cst")
            nc.sync.dma_start(out=cst[:], in_=d_cst.ap())
            mskp = cpool.tile([P, BPC * NC_CHUNKS], F32, name="mskp")
            nc.sync.dma_start(out=mskp[:], in_=d_msk.ap())

            M1s = [cst[:, COFF[c]:COFF[c] + CW[c]] for c in range(NC_CHUNKS)]
            DsB = [cst[:, DOFF + c * P:DOFF + (c + 1) * P]
                   for c in range(NC_CHUNKS)]

            def build_S(b):
                # S_b[j,l] = M1s[j,l]*mask[j] + Ds[j,l], causal-compact:
                # tile column i == original column 128*mc + i
                msk = mskp[:, b * NC_CHUNKS:(b + 1) * NC_CHUNKS]
                Sl = []
                for mc in range(NC_CHUNKS):
                    st = apool.tile([P, CW[mc]], F32, name=f"s{b}_{mc}",
                                    tag=f"s{mc}")
                    nc.vector.scalar_tensor_tensor(
                        out=st[:, :P],
                        in0=M1s[mc][:, :P],
                        scalar=msk[:, mc:mc + 1],
                        in1=DsB[mc][:],
                        op0=OP.mult,
                        op1=OP.add,
                    )
                    if mc < NC_CHUNKS - 1:
                        nc.vector.tensor_scalar_mul(
                            out=st[:, P:],
                            in0=M1s[mc][:, P:],
                            scalar1=msk[:, mc:mc + 1],
                        )
                    Sl.append(st)
                return Sl

            def z_group(bb, XTb, kc_major=False):
                # Z^T[h,l] = silu(sum_k Wz[h,k] XT[k,l]).  kc-major order (all
                # hc banks per contraction chunk) lets item 0 start as soon as
                # the first x/Wz chunks land instead of waiting for all four.
                zps = [pspool.tile([P, L], F32, name=f"zp{bb}_{hc}", tag="mm")
                       for hc in range(NC_CHUNKS)]

                def mm(hc, kc):
                    nc.tensor.matmul(
                        out=zps[hc][:],
                        lhsT=WzT[kc][:, hc * P:(hc + 1) * P],
                        rhs=XTb[kc],
                        start=(kc == 0),
                        stop=(kc == NC_CHUNKS - 1),
                    )

                order = ([(hc, kc) for kc in range(NC_CHUNKS)
                          for hc in range(NC_CHUNKS)] if kc_major else
                         [(hc, kc) for hc in range(NC_CHUNKS)
                          for kc in range(NC_CHUNKS)])
                for hc, kc in order:
                    mm(hc, kc)
                ZTl = []
                for hc in range(NC_CHUNKS):
                    zt = apool.tile([P, L], BF16, name=f"zt{bb}_{hc}",
                                    tag=f"zt{hc}")
                    nc.scalar.activation(out=zt[:], in_=zps[hc][:], func=AF.Silu)
                    ZTl.append(zt)
                return ZTl

            ZT_next = z_group(0, XT_tiles[0], kc_major=True)

            for b in range(BPC):
                XT = XT_tiles[b]
                ZT = ZT_next

                # ---- V[l,h] = silu(sum_k XT[k,l] WvT[k,h]) ----
                def v_chunk(lc):
                    # item 0: borrow the (mid-kernel idle) tppool banks so V
                    # never waits on the bunched kc-major Z evacuations -- a
                    # PE stall there would reset the clock ramp
                    if b == 0 and lc < 3:
                        vp = tppool.tile([P, L], F32, name=f"vp{b}_{lc}",
                                         tag=f"tp{lc + 1}")
                    else:
                        vp = pspool.tile([P, L], F32, name=f"vp{b}_{lc}",
                                         tag="mm")
                    for kc in range(NC_CHUNKS):
                        nc.tensor.matmul(
                            out=vp[:],
                            lhsT=XT[kc][:, lc * P:(lc + 1) * P],
                            rhs=WvT[kc],
                            start=(kc == 0),
                            stop=(kc == NC_CHUNKS - 1),
                        )
                    vt = apool.tile([P, L], BF16, name=f"v{b}_{lc}",
                                    tag=f"v{lc}")
                    nc.scalar.activation(out=vt[:], in_=vp[:], func=AF.Silu)
                    return vt

                # ---- Y^T = Wqk^T-contracted Z^T (Q@K^T folded; see top) ----
                def y_group():
                    YTl = []
                    for hc in range(NC_CHUNKS):
                        yp = pspool.tile([P, L], F32, name=f"yp{b}_{hc}",
                                         tag="mm")
                        for kc in range(NC_CHUNKS):
                            nc.tensor.matmul(
                                out=yp[:],
                                lhsT=Wqk[kc][:, hc * P:(hc + 1) * P],
                                rhs=ZT[kc][:],
                                start=(kc == 0),
                                stop=(kc == NC_CHUNKS - 1),
                            )
                        yt = apool.tile([P, L], BF16, name=f"yt{b}_{hc}",
                                        tag=f"yt{hc}")
                        # DVE evac: the ACT queue's silu backlog would delay
                        # YT[3] and stall the first P chunk's last contraction
                        nc.vector.tensor_copy(out=yt[:], in_=yp[:])
                        YTl.append(yt)
                    return YTl

                if b < BPC - 1:
                    V = [v_chunk(lc) for lc in range(NC_CHUNKS)]
                    emit_xt(b + 1)
                    YT = y_group()
                else:
                    # Tail item: Y first so the P->A->OUT chain starts ~850ns
                    # sooner; V fills the PE while the A chunks are built
                    YT = y_group()
                    V = [v_chunk(0)]

                # S for this item was built one item ahead (below), so its
                # DVE ops are never stuck behind u's head-of-line PSUM waits
                S = S_next if b > 0 else build_S(0)
                if b + 1 < BPC:
                    S_next = build_S(b + 1)

                # ---- P^T[m,l] = sum_e ZT[e,m] YT[e,l] ; A^T = (relu*S)^2 ----
                def p_chunk(mc):
                    ms = mc * P
                    # item 0: tppool banks again, so item 1's Z matmuls reuse
                    # banks released by ACT evacs instead of waiting on the
                    # DVE's read of pp(0,*)
                    if b == 0 and mc < 3:
                        pp = tppool.tile([P, L], F32, name=f"pp{b}_{mc}",
                                         tag=f"tp{mc + 1}")
                    else:
                        pp = pspool.tile([P, L], F32, name=f"pp{b}_{mc}",
                                         tag="mm")
                    for dc in range(NC_CHUNKS):
                        nc.tensor.matmul(
                            out=pp[:, :CW[mc]],
                            lhsT=ZT[dc][:, ms:ms + P],
                            rhs=YT[dc][:, ms:],
                            start=(dc == 0),
                            stop=(dc == NC_CHUNKS - 1),
                        )
                    return pp

                def a_chunk(mc, pp, sq_eng, fresh=False):
                    # fresh=True gives the tail item its own buffers so no
                    # WAR wait on earlier items' consumers blocks the DVE
                    utag = f"uL{mc}" if fresh else "u"
                    u = smpool.tile([P, CW[mc]], F32, name=f"u{b}_{mc}",
                                    tag=utag)
                    nc.vector.scalar_tensor_tensor(
                        out=u[:],
                        in0=pp[:, :CW[mc]],
                        scalar=0.0,
                        in1=S[mc][:],
                        op0=OP.max,
                        op1=OP.mult,
                    )
                    atag = f"aL{mc}" if fresh else f"a{mc}"
                    at = apool.tile([P, CW[mc]], BF16, name=f"a{b}_{mc}",
                                    tag=atag)
                    if sq_eng == "act":
                        nc.scalar.square(out=at[:], in_=u[:])
                    else:
                        nc.vector.tensor_mul(out=at[:], in0=u[:], in1=u[:])
                    return at

                def emit_out_pair(pc, ops):
                    # two [128,512] blocks -> one [256,512] DMA on the Pool
                    # queue (descriptor-gen off the shared HWDGE device)
                    ot = smpool.tile([P, 2 * L], BF16, name=f"o{b}_{pc}",
                                     tag="o", bufs=6)
                    nc.vector.tensor_copy(out=ot[:, :L], in_=ops[0][:])
                    nc.vector.tensor_copy(out=ot[:, L:], in_=ops[1][:])
                    nc.gpsimd.dma_start(
                        out=d_out.ap()[b, pc * 2 * P:(pc + 1) * 2 * P, :]
                        .rearrange("(c p) j -> p c j", p=P),
                        in_=ot[:],
                    )

                if b < BPC - 1:
                    A = [a_chunk(mc, p_chunk(mc), "act")
                         for mc in range(NC_CHUNKS)]
                    # next batch's Z matmuls fill the PE while ACT/DVE build A
                    ZT_next = z_group(b + 1, XT_tiles[b + 1])
                    # ---- OUT[l,h] = sum_m A[m,l] V[m,h] ----
                    ops = []
                    for lc in range(NC_CHUNKS):
                        op_ = pspool.tile([P, L], F32, name=f"op{b}_{lc}",
                                          tag="mm")
                        for mc in range(lc + 1):  # A[mc] is 0 for mc > lc
                            nc.tensor.matmul(
                                out=op_[:],
                                lhsT=A[mc][:, (lc - mc) * P:(lc - mc + 1) * P],
                                rhs=V[mc][:],
                                start=(mc == 0),
                                stop=(mc == lc),
                            )
                        ops.append(op_)
                        if lc % 2 == 1:
                            emit_out_pair(lc // 2, ops[-2:])
                else:
                    # Tail item: all P chunks run right after Y; the remaining
                    # V chunks then fill the PE while the DVE builds every A
                    # chunk (u and square back-to-back, fresh buffers, no WAR
                    # waits), so the OUT matmuls run gap-free and the kernel
                    # tail after the last matmul is pure evac+DMA latency.
                    # Output blocks drain individually in completion order;
                    # lc0 rides the Pool DMA queue, the rest take the
                    # (tail-idle) HWDGE path.  The tppool PSUM banks are free
                    # here (warmup long done).
                    Ops = [
                        tppool.tile([P, L], F32, name=f"opL_{lc}",
                                    tag=f"tp{lc}")
                        for lc in range(NC_CHUNKS)
                    ]
                    pps = [p_chunk(mc) for mc in range(NC_CHUNKS)]
                    V += [v_chunk(lc) for lc in range(1, NC_CHUNKS)]
                    A = {mc: a_chunk(mc, pps[mc], "dve", fresh=True)
                         for mc in range(NC_CHUNKS)}

                    def evac_block(lc):
                        # fresh tags: a rotating buffer here would WAR-wait on
                        # an old output pair's DMA (+900ns sem propagation)
                        ot = smpool.tile([P, L], BF16, name=f"oL_{lc}",
                                         tag=f"oL{lc}", bufs=1)
                        nc.scalar.copy(out=ot[:], in_=Ops[lc][:])
                        return ot

                    def dma_block(lc, ot, queue):
                        queue.dma_start(
                            out=d_out.ap()[b, lc * P:(lc + 1) * P, :],
                            in_=ot[:],
                        )

                    # lc-major completion order; accumulation into each Ops[lc]
                    # stays mc-ascending as required by start/stop flags.
                    # The four drains issue from three different queues (the
                    # 565ns per-DMA sequencer config would otherwise pace
                    # them); lc2's scalar-queue DMA is emitted after lc3's
                    # evacuation so it doesn't block it on the ACT sequencer.
                    ots = {}
                    for lc in range(NC_CHUNKS):
                        for mc in range(lc + 1):
                            nc.tensor.matmul(
                                out=Ops[lc][:],
                                lhsT=A[mc][:, (lc - mc) * P:(lc - mc + 1) * P],
                                rhs=V[mc][:],
                                start=(mc == 0),
                                stop=(mc == lc),
                            )
                        ots[lc] = evac_block(lc)
                        if lc == 0:
                            dma_block(0, ots[0], nc.sync)
                        elif lc == 1:
                            dma_block(1, ots[1], nc.gpsimd)
                        elif lc == 3:
                            dma_block(2, ots[2], nc.scalar)
                            dma_block(3, ots[3], nc.sync)

    nc.compile()
    return nc


def _host_prep(positives, mask, item_emb, pos_emb, Wz, Wv, Wq, Wk,
               gamma_q, beta_q, gamma_k, beta_k, sparse_w, gumbel):
    """Host-side constant folding + input staging + per-core shards."""
    f32 = np.float32
    bf16 = ml_dtypes.bfloat16
    positives = np.asarray(positives)
    maskf = np.asarray(mask).astype(f32)
    item_emb = np.asarray(item_emb, f32)
    pos_emb = np.asarray(pos_emb, f32)
    sw = np.asarray(sparse_w, f32)
    gum = np.asarray(gumbel, f32)

    smask = (1.0 / (1.0 + np.exp(-((np.log(sw / (1.0 - sw)) + gum) / f32(TEMP)))))
    smask = smask.astype(f32)
    scl = f32(1.0 / np.sqrt(L * H))
    j = np.arange(L)
    strict_lower_T = (j[:, None] < j[None, :])  # [j, l] : j < l
    M1s_mat = (smask.T * strict_lower_T * scl).astype(f32)
    dsv = (np.diag(smask) * scl).astype(f32)

    # one [128, CST_COLS] tile: causal-compacted M1s chunks + diag blocks
    cstp = np.zeros((P, CST_COLS), f32)
    for c in range(NC_CHUNKS):
        cstp[:, COFF[c]:COFF[c] + CW[c]] = M1s_mat[c * P:(c + 1) * P, c * P:]
        blk = np.zeros((P, P), f32)
        np.fill_diagonal(blk, dsv[c * P:(c + 1) * P])
        cstp[:, DOFF + c * P:DOFF + (c + 1) * P] = blk
    cstp = np.ascontiguousarray(cstp.astype(bf16))

    # Q@K^T folds to Z (Wq^T diag(gq*gk) Wk) Z^T only when both betas vanish
    # (true for this model's inputs); fail loudly rather than silently wrong.
    assert not np.any(np.asarray(beta_q)) and not np.any(np.asarray(beta_k)), (
        "kernel assumes beta_q == beta_k == 0 (holds for setup_inputs)"
    )
    g = np.asarray(gamma_q, np.float64) * np.asarray(gamma_k, np.float64)
    Wqk = (np.asarray(Wq, np.float64).T
           @ (g[:, None] * np.asarray(Wk, np.float64))).astype(f32)

    # host staging: XT_b = (emb[pos_b] + pos_emb)^T, chunked [4,128,L] bf16
    X = item_emb[positives] + pos_emb[None]                  # [B, L, H] f32
    XT = np.ascontiguousarray(X.transpose(0, 2, 1)).astype(bf16)
    XT = XT.reshape(B, NC_CHUNKS, P, L)

    # mask packed [128, BPC*4]: mskp[p, b*4+c] = mask[b, c*128+p]
    m4 = maskf.reshape(B, NC_CHUNKS, P)

    shared = {
        "WzT": np.ascontiguousarray(np.asarray(Wz, f32).T.astype(bf16)),
        "WvT": np.ascontiguousarray(np.asarray(Wv, f32).T.astype(bf16)),
        "Wqk": np.ascontiguousarray(Wqk.astype(bf16)),
        "cst": cstp,
    }
    in_maps = []
    for c in range(N_CORES):
        sl = slice(c * BPC, (c + 1) * BPC)
        m = dict(shared)
        m["XT"] = np.ascontiguousarray(XT[sl])
        m["mskp"] = np.ascontiguousarray(
            m4[sl].transpose(2, 0, 1).reshape(P, BPC * NC_CHUNKS)
        )
        in_maps.append(m)
    return in_maps


def get_module():
    global _COMPILED
    if _COMPILED is None:
        _COMPILED = _build_module()
    return _COMPILED


def kernel(**inputs) -> np.ndarray:
    nc = get_module()
    in_maps = _host_prep(**inputs)
    res = run_bass_kernel_spmd(nc, in_maps, core_ids=list(range(N_CORES)))
    out = np.concatenate([r["out"] for r in res.results], axis=0)
    return out.astype(np.float32)


if __name__ == "__main__":
    rng = np.random.default_rng(0)
    demo = {
        "positives": rng.integers(0, ITEM, (B, L)).astype(np.int32),
        "mask": rng.integers(0, 2, (B, L)).astype(np.int32),
        "item_emb": rng.normal(size=(ITEM, H)).astype(np.float32) * 0.02,
        "pos_emb": rng.normal(size=(L, H)).astype(np.float32) * 0.02,
        "Wz": rng.normal(size=(L, L)).astype(np.float32),
        "Wv": rng.normal(size=(L, L)).astype(np.float32),
        "Wq": rng.normal(size=(L, L)).astype(np.float32),
        "Wk": rng.normal(size=(L, L)).astype(np.float32),
        "gamma_q": rng.normal(size=(L,)).astype(np.float32) * 0.02,
        "beta_q": np.zeros((L,), np.float32),
        "gamma_k": rng.normal(size=(L,)).astype(np.float32) * 0.02,
        "beta_k": np.zeros((L,), np.float32),
        "sparse_w": rng.uniform(0.2, 0.8, (L, H)).astype(np.float32),
        "gumbel": rng.normal(size=(L, H)).astype(np.float32),
    }
    out = kernel(**demo)
    print("out", out.shape, out.dtype, np.abs(out).max())


# revision 39
# speedup vs baseline: 1.0015x; 1.0002x over previous
"""Trainium2 Bass kernel for nn_DenoisedSasrec (GAU-style sparse attention).

Contract: kernel(**inputs) takes FULL unsharded numpy inputs (as produced by
setup_inputs) and returns the FULL [64, 512, 512] float32 output.

Strategy (data-parallel over batch, per sharding hint):
  - 64 batch items are sharded 8-per-core across the 8 NeuronCores.
  - Projection weights and the [L,L] sparse-mask constants are replicated
    to every core; the embedding gather, the pos_emb add AND the transpose
    are all folded into host input staging, so the device receives
    XT_b = (item_emb[pos_b] + pos_emb)^T directly and spends zero PE cycles
    on transposes (PE work is the hard floor of this kernel).
  - Per batch item, on device:
      Z^T = silu(Wz @ X^T), V = silu(X @ Wv^T)    (PE + ACT)
      Y^T = Wqk-contracted Z^T                    (PE + ACT copy)
      P^T = Z^T-contracted attention logits       (PE)
      A^T = (relu(P^T) * S_b)^2                   (DVE relu*mask, ACT square)
      OUT = A @ V                                 (PE)
  with Wqk = Wq^T diag(gamma_q*gamma_k) Wk folded on host (exact when
  beta_q == beta_k == 0, which holds for this model's inputs), so Q@K^T
  costs one GEMM instead of two and Z^T doubles as the attention lhsT.
  S_b[j,l] = smask[l,j]*keep_b[l,j]/sqrt(L*H) is built per batch from a
  host-packed constant tile (causal-truncated M1s chunks + expanded diag
  blocks in ONE DMA) and the per-key padding mask; the mask/smask/relu^2/
  (L*H) algebra folds exactly into (relu(P)*S)^2 because smask>0 and keep
  is 0/1.

  Performance notes (TRN2 cost-model facts this schedule is built around:
  matmuls cost output-free-size cycles; every HWDGE DMA serializes a
  ~625ns slot on one global descriptor device while Pool-queue DMAs
  desc-gen on the Pool engine instead; transfers serialize on one global
  DMA device; every DMA completion semaphore takes +900ns to propagate;
  the PE clock ramps 0.65->1.2->2.4GHz with 3us of CONTINUOUS busy and
  any idle gap resets it; each DMA also holds its issuing sequencer
  ~565ns):
  - bf16 operands with fp32 PSUM accumulation everywhere (rel err ~7e-3,
    gate 2e-2); bf16 streams the PE at 1 row/cycle at any width.
  - Attention is causal: for key-chunk mc, columns l < 128*mc of A^T are
    exactly zero, so P/A/S tiles shrink to the live range and 6 of 16 OUT
    matmuls per item are skipped - exact, no approximation.
  - 20 narrow + 1 wide warmup matmuls on memset tiles ramp the PE to
    2.4GHz and end exactly when the first x/Wz chunks land (~3.7us, two
    parallel DMA paths), so every real GEMM runs at full clock with the
    PE >99.5% busy between first and last matmul.
  - Per-item stages are software-pipelined to keep the PE queue dense:
    item b+1's Z GEMMs are emitted inside item b's attention phase; S
    masks are built one item ahead; item 0 borrows the tail-only PSUM
    banks for V/P so no bank-reuse WAR ever stalls the PE.
  - The tail item runs Y before V, then all P chunks, then the remaining
    V chunks while the DVE squares every A chunk, so the final OUT
    matmuls run gap-free and the kernel tail is pure evac+DMA latency,
    spread across the sync/pool/scalar DMA queues.
  - All mask/M1s/diag constants ship in 2 DMAs; X^T ships 1 pool DMA per
    item (items 0/1 as sync chunks to cut time-to-first-matmul); outputs
    ship as 256-row pairs (2 pool DMAs/item) except the tail item's
    blocks which go individually for latency.
  - Output is written bf16 and upcast on host (halves output DMA).
"""

import numpy as np
import ml_dtypes

import concourse.bass as bass
import concourse.mybir as mybir
import concourse.tile as tile
from concourse import bacc
from concourse.bass_utils import run_bass_kernel_spmd

B, L, H = 64, 512, 512
ITEM = 50001
TEMP = 0.2
N_CORES = 8
BPC = B // N_CORES  # batches per core
P = 128
NC_CHUNKS = L // P  # 4
# Warmup matmuls ramp the PE clock before the first input DMA lands
# (~4.04us: pool desc-gen + DGE delay + transfer + 900ns sem propagation).
# 24 narrow matmuls (PE.SEQ-paced, ~116ns each) plus one wide one land the
# last warmup at ~data-ready with no sequencer backlog, so the first real
# matmul starts immediately and the whole kernel runs at 2.4GHz.
N_WARM = 18

# causal-truncated widths / pack offsets for the M1s+diag constant tile
CW = [L - c * P for c in range(NC_CHUNKS)]        # 512,384,256,128
COFF = [0, 512, 896, 1152]
DOFF = 1280                                       # diag blocks at the end
CST_COLS = DOFF + NC_CHUNKS * P                   # 1792

F32 = mybir.dt.float32
BF16 = mybir.dt.bfloat16

_COMPILED = None  # cache (nc) across calls


def _build_module():
    nc = bacc.Bacc("TRN2", target_bir_lowering=False, debug=False)

    # ---- DRAM I/O ----
    d_xt = nc.dram_tensor("XT", [BPC, NC_CHUNKS, P, L], BF16,
                          kind="ExternalInput")
    d_msk = nc.dram_tensor("mskp", [P, BPC * NC_CHUNKS], F32,
                           kind="ExternalInput")
    d_wzt = nc.dram_tensor("WzT", [H, H], BF16, kind="ExternalInput")
    d_wvt = nc.dram_tensor("WvT", [H, H], BF16, kind="ExternalInput")
    d_wqk = nc.dram_tensor("Wqk", [H, H], BF16, kind="ExternalInput")
    d_cst = nc.dram_tensor("cst", [P, CST_COLS], BF16, kind="ExternalInput")
    d_out = nc.dram_tensor("out", [BPC, L, H], BF16, kind="ExternalOutput")

    AF = mybir.ActivationFunctionType
    OP = mybir.AluOpType

    with tile.TileContext(nc) as tc:
        with (
            tc.tile_pool(name="const", bufs=1) as cpool,
            tc.tile_pool(name="io", bufs=2) as iopool,
            tc.tile_pool(name="acts", bufs=2) as apool,
            tc.tile_pool(name="small", bufs=3) as smpool,
            tc.tile_pool(name="psum", bufs=4, space="PSUM") as pspool,
            tc.tile_pool(name="psumt", bufs=1, space="PSUM") as tppool,
        ):
            # ---- PE warmup: ramp the clock while the first DMAs fly.  A
            # tiny DVE memset (fastest-starting engine) feeds narrow matmuls
            # into a dead PSUM bank that real work later overwrites. ----
            wsrc = cpool.tile([P, P], BF16, name="wsrc")
            nc.vector.memset(wsrc[:], 0.0)
            wsrcw = cpool.tile([P, L], BF16, name="wsrcw")
            nc.vector.memset(wsrcw[:], 0.0)
            wp = tppool.tile([P, L], F32, name="warm", tag="tp0")
            for _ in range(N_WARM):
                nc.tensor.matmul(out=wp[:, :P], lhsT=wsrc[:], rhs=wsrc[:],
                                 start=True, stop=True)
            nc.tensor.matmul(out=wp[:], lhsT=wsrc[:], rhs=wsrcw[:],
                             start=True, stop=True)

            # ---- input DMAs (HWDGE order == emission order on sync) ----
            def load_w(dram, name, quarters=False):
                # quarters=True: four [128,512] DMAs, so the first contraction
                # chunks land (and clear the shared DMA-transfer device) early
                n = 4 if quarters else 2
                rows = H // n                     # 128-row groups per DMA
                cpd = rows // P                   # kc chunks per DMA tile
                tiles = [cpool.tile([P, cpd * L], BF16, name=f"{name}h{h}")
                         for h in range(n)]

                def emit(h, queue=None):
                    (queue or nc.sync).dma_start(
                        out=tiles[h][:],
                        in_=dram.ap()[h * rows:(h + 1) * rows]
                        .rearrange("(c p) j -> p c j", p=P),
                    )
                views = [tiles[kc // cpd][:, (kc % cpd) * L:(kc % cpd + 1) * L]
                         for kc in range(NC_CHUNKS)]
                return views, emit

            XT_tiles = {}

            # x loads ride the Pool/SWDGE queue: descriptor-gen runs on the
            # otherwise-idle Pool engine instead of the globally-serialized
            # HWDGE device, so weight and x loads land in parallel paths
            def emit_xt(b):
                xw = iopool.tile([P, NC_CHUNKS * L], BF16, name=f"x{b}",
                                 tag="xw")
                src = d_xt.ap()[b].rearrange("c p j -> p c j")
                if b == 1:
                    # keep x1 off the transfer device until the item-0
                    # weight loads have cleared it: four small sync DMAs
                    # (emitted after msk) instead of one big early pool DMA
                    for kc in range(NC_CHUNKS):
                        nc.sync.dma_start(out=xw[:, kc * L:(kc + 1) * L],
                                          in_=src[:, kc:kc + 1])
                else:
                    nc.gpsimd.dma_start(out=xw[:], in_=src)
                XT_tiles[b] = [xw[:, kc * L:(kc + 1) * L]
                               for kc in range(NC_CHUNKS)]

            WzT, wz_emit = load_w(d_wzt, "wz", quarters=True)
            WvT, wv_emit = load_w(d_wvt, "wv")
            Wqk, wqk_emit = load_w(d_wqk, "wqk")
            # hand-tuned two-path schedule for item 0: Wz quarters 0/2/3 ride
            # Pool descriptor-gen while the x0 chunks + Wz quarter 1 take
            # HWDGE, interleaved so the shared DMA-transfer device serves the
            # kc-major Z groups just in time
            x0 = iopool.tile([P, NC_CHUNKS * L], BF16, name="x0", tag="xw")
            x0src = d_xt.ap()[0].rearrange("c p j -> p c j")

            def x0_emit(kc):
                nc.sync.dma_start(out=x0[:, kc * L:(kc + 1) * L],
                                  in_=x0src[:, kc:kc + 1])

            wz_emit(0, nc.gpsimd)
            x0_emit(0)
            wz_emit(1, nc.sync)
            x0_emit(1)
            x0_emit(2)
            x0_emit(3)
            wz_emit(2, nc.gpsimd)
            wz_emit(3, nc.gpsimd)
            XT_tiles[0] = [x0[:, kc * L:(kc + 1) * L]
                           for kc in range(NC_CHUNKS)]
            wv_emit(0)
            wv_emit(1)
            wqk_emit(0)
            wqk_emit(1)
            cst = cpool.tile([P, CST_COLS], BF16, name="cst")
            nc.sync.dma_start(out=cst[:], in_=d_cst.ap())
            mskp = cpool.tile([P, BPC * NC_CHUNKS], F32, name="mskp")
            nc.sync.dma_start(out=mskp[:], in_=d_msk.ap())

            M1s = [cst[:, COFF[c]:COFF[c] + CW[c]] for c in range(NC_CHUNKS)]
            DsB = [cst[:, DOFF + c * P:DOFF + (c + 1) * P]
                   for c in range(NC_CHUNKS)]

            def build_S(b):
                # S_b[j,l] = M1s[j,l]*mask[j] + Ds[j,l], causal-compact:
                # tile column i == original column 128*mc + i
                msk = mskp[:, b * NC_CHUNKS:(b + 1) * NC_CHUNKS]
                Sl = []
                for mc in range(NC_CHUNKS):
                    st = apool.tile([P, CW[mc]], F32, name=f"s{b}_{mc}",
                                    tag=f"s{mc}")
                    nc.vector.scalar_tensor_tensor(
                        out=st[:, :P],
                        in0=M1s[mc][:, :P],
                        scalar=msk[:, mc:mc + 1],
                        in1=DsB[mc][:],
                        op0=OP.mult,
                        op1=OP.add,
                    )
                    if mc < NC_CHUNKS - 1:
                        nc.vector.tensor_scalar_mul(
                            out=st[:, P:],
                            in0=M1s[mc][:, P:],
                            scalar1=msk[:, mc:mc + 1],
                        )
                    Sl.append(st)
                return Sl

            def z_group(bb, XTb, kc_major=False):
                # Z^T[h,l] = silu(sum_k Wz[h,k] XT[k,l]).  kc-major order (all
                # hc banks per contraction chunk) lets item 0 start as soon as
                # the first x/Wz chunks land instead of waiting for all four.
                zps = [pspool.tile([P, L], F32, name=f"zp{bb}_{hc}", tag="mm")
                       for hc in range(NC_CHUNKS)]

                def mm(hc, kc):
                    nc.tensor.matmul(
                        out=zps[hc][:],
                        lhsT=WzT[kc][:, hc * P:(hc + 1) * P],
                        rhs=XTb[kc],
                        start=(kc == 0),
                        stop=(kc == NC_CHUNKS - 1),
                    )

                order = ([(hc, kc) for kc in range(NC_CHUNKS)
                          for hc in range(NC_CHUNKS)] if kc_major else
                         [(hc, kc) for hc in range(NC_CHUNKS)
                          for kc in range(NC_CHUNKS)])
                for hc, kc in order:
                    mm(hc, kc)
                ZTl = []
                for hc in range(NC_CHUNKS):
                    zt = apool.tile([P, L], BF16, name=f"zt{bb}_{hc}",
                                    tag=f"zt{hc}")
                    nc.scalar.activation(out=zt[:], in_=zps[hc][:], func=AF.Silu)
                    ZTl.append(zt)
                return ZTl

            ZT_next = z_group(0, XT_tiles[0], kc_major=True)

            for b in range(BPC):
                XT = XT_tiles[b]
                ZT = ZT_next

                # ---- V[l,h] = silu(sum_k XT[k,l] WvT[k,h]) ----
                def v_chunk(lc):
                    # item 0: borrow the (mid-kernel idle) tppool banks so V
                    # never waits on the bunched kc-major Z evacuations -- a
                    # PE stall there would reset the clock ramp
                    if b == 0 and lc < 3:
                        vp = tppool.tile([P, L], F32, name=f"vp{b}_{lc}",
                                         tag=f"tp{lc + 1}")
                    else:
                        vp = pspool.tile([P, L], F32, name=f"vp{b}_{lc}",
                                         tag="mm")
                    for kc in range(NC_CHUNKS):
                        nc.tensor.matmul(
                            out=vp[:],
                            lhsT=XT[kc][:, lc * P:(lc + 1) * P],
                            rhs=WvT[kc],
                            start=(kc == 0),
                            stop=(kc == NC_CHUNKS - 1),
                        )
                    vt = apool.tile([P, L], BF16, name=f"v{b}_{lc}",
                                    tag=f"v{lc}")
                    nc.scalar.activation(out=vt[:], in_=vp[:], func=AF.Silu)
                    return vt

                # ---- Y^T = Wqk^T-contracted Z^T (Q@K^T folded; see top) ----
                def y_group():
                    YTl = []
                    for hc in range(NC_CHUNKS):
                        yp = pspool.tile([P, L], F32, name=f"yp{b}_{hc}",
                                         tag="mm")
                        for kc in range(NC_CHUNKS):
                            nc.tensor.matmul(
                                out=yp[:],
                                lhsT=Wqk[kc][:, hc * P:(hc + 1) * P],
                                rhs=ZT[kc][:],
                                start=(kc == 0),
                                stop=(kc == NC_CHUNKS - 1),
                            )
                        yt = apool.tile([P, L], BF16, name=f"yt{b}_{hc}",
                                        tag=f"yt{hc}")
                        # DVE evac: the ACT queue's silu backlog would delay
                        # YT[3] and stall the first P chunk's last contraction
                        nc.vector.tensor_copy(out=yt[:], in_=yp[:])
                        YTl.append(yt)
                    return YTl

                if b < BPC - 1:
                    V = [v_chunk(lc) for lc in range(NC_CHUNKS)]
                    emit_xt(b + 1)
                    YT = y_group()
                else:
                    # Tail item: Y first so the P->A->OUT chain starts ~850ns
                    # sooner; V fills the PE while the A chunks are built
                    YT = y_group()
                    V = [v_chunk(0)]

                # S for this item was built one item ahead (below), so its
                # DVE ops are never stuck behind u's head-of-line PSUM waits
                S = S_next if b > 0 else build_S(0)
                if b + 1 < BPC:
                    S_next = build_S(b + 1)

                # ---- P^T[m,l] = sum_e ZT[e,m] YT[e,l] ; A^T = (relu*S)^2 ----
                def p_chunk(mc):
                    ms = mc * P
                    # item 0: tppool banks again, so item 1's Z matmuls reuse
                    # banks released by ACT evacs instead of waiting on the
                    # DVE's read of pp(0,*)
                    if b == 0 and mc < 3:
                        pp = tppool.tile([P, L], F32, name=f"pp{b}_{mc}",
                                         tag=f"tp{mc + 1}")
                    else:
                        pp = pspool.tile([P, L], F32, name=f"pp{b}_{mc}",
                                         tag="mm")
                    for dc in range(NC_CHUNKS):
                        nc.tensor.matmul(
                            out=pp[:, :CW[mc]],
                            lhsT=ZT[dc][:, ms:ms + P],
                            rhs=YT[dc][:, ms:],
                            start=(dc == 0),
                            stop=(dc == NC_CHUNKS - 1),
                        )
                    return pp

                def a_chunk(mc, pp, sq_eng, fresh=False):
                    # fresh=True gives the tail item its own buffers so no
                    # WAR wait on earlier items' consumers blocks the DVE
                    utag = f"uL{mc}" if fresh else "u"
                    u = smpool.tile([P, CW[mc]], F32, name=f"u{b}_{mc}",
                                    tag=utag)
                    nc.vector.scalar_tensor_tensor(
                        out=u[:],
                        in0=pp[:, :CW[mc]],
                        scalar=0.0,
                        in1=S[mc][:],
                        op0=OP.max,
                        op1=OP.mult,
                    )
                    atag = f"aL{mc}" if fresh else f"a{mc}"
                    at = apool.tile([P, CW[mc]], BF16, name=f"a{b}_{mc}",
                                    tag=atag)
                    if sq_eng == "act":
                        nc.scalar.square(out=at[:], in_=u[:])
                    else:
                        nc.vector.tensor_mul(out=at[:], in0=u[:], in1=u[:])
                    return at

                def emit_out_pair(pc, ops):
                    # two [128,512] blocks -> one [256,512] DMA on the Pool
                    # queue (descriptor-gen off the shared HWDGE device)
                    ot = smpool.tile([P, 2 * L], BF16, name=f"o{b}_{pc}",
                                     tag="o", bufs=6)
                    nc.vector.tensor_copy(out=ot[:, :L], in_=ops[0][:])
                    nc.vector.tensor_copy(out=ot[:, L:], in_=ops[1][:])
                    nc.gpsimd.dma_start(
                        out=d_out.ap()[b, pc * 2 * P:(pc + 1) * 2 * P, :]
                        .rearrange("(c p) j -> p c j", p=P),
                        in_=ot[:],
                    )

                if b < BPC - 1:
                    A = [a_chunk(mc, p_chunk(mc), "act")
                         for mc in range(NC_CHUNKS)]
                    # next batch's Z matmuls fill the PE while ACT/DVE build A
                    ZT_next = z_group(b + 1, XT_tiles[b + 1])
                    # ---- OUT[l,h] = sum_m A[m,l] V[m,h] ----
                    ops = []
                    for lc in range(NC_CHUNKS):
                        op_ = pspool.tile([P, L], F32, name=f"op{b}_{lc}",
                                          tag="mm")
                        for mc in range(lc + 1):  # A[mc] is 0 for mc > lc
                            nc.tensor.matmul(
                                out=op_[:],
                                lhsT=A[mc][:, (lc - mc) * P:(lc - mc + 1) * P],
                                rhs=V[mc][:],
                                start=(mc == 0),
                                stop=(mc == lc),
                            )
                        ops.append(op_)
                        if lc % 2 == 1:
                            emit_out_pair(lc // 2, ops[-2:])
                else:
                    # Tail item: all P chunks run right after Y; the remaining
                    # V chunks then fill the PE while the DVE builds every A
                    # chunk (u and square back-to-back, fresh buffers, no WAR
                    # waits), so the OUT matmuls run gap-free and the kernel
                    # tail after the last matmul is pure evac+DMA latency.
                    # Output blocks drain individually in completion order;
                    # lc0 rides the Pool DMA queue, the rest take the
                    # (tail-idle) HWDGE path.  The tppool PSUM banks are free
                    # here (warmup long done).
                    Ops = [
                        tppool.tile([P, L], F32, name=f"opL_{lc}",
                                    tag=f"tp{lc}")
                        for lc in range(NC_CHUNKS)
                    ]
                    pps = [p_chunk(mc) for mc in range(NC_CHUNKS)]
                    V += [v_chunk(lc) for lc in range(1, NC_CHUNKS)]
                    A = {mc: a_chunk(mc, pps[mc], "dve", fresh=True)
                         for mc in range(NC_CHUNKS)}

                    def evac_block(lc):
                        # fresh tags: a rotating buffer here would WAR-wait on
                        # an old output pair's DMA (+900ns sem propagation)
                        ot = smpool.tile([P, L], BF16, name=f"oL_{lc}",
                                         tag=f"oL{lc}", bufs=1)
                        nc.scalar.copy(out=ot[:], in_=Ops[lc][:])
                        return ot

                    def dma_block(lc, ot, queue):
                        queue.dma_start(
                            out=d_out.ap()[b, lc * P:(lc + 1) * P, :],
                            in_=ot[:],
                        )

                    # lc-major completion order; accumulation into each Ops[lc]
                    # stays mc-ascending as required by start/stop flags.
                    # The four drains issue from three different queues (the
                    # 565ns per-DMA sequencer config would otherwise pace
                    # them); lc2's scalar-queue DMA is emitted after lc3's
                    # evacuation so it doesn't block it on the ACT sequencer.
                    ots = {}
                    for lc in range(NC_CHUNKS):
                        for mc in range(lc + 1):
                            nc.tensor.matmul(
                                out=Ops[lc][:],
                                lhsT=A[mc][:, (lc - mc) * P:(lc - mc + 1) * P],
                                rhs=V[mc][:],
                                start=(mc == 0),
                                stop=(mc == lc),
                            )
                        ots[lc] = evac_block(lc)
                        if lc == 0:
                            dma_block(0, ots[0], nc.sync)
                        elif lc == 1:
                            dma_block(1, ots[1], nc.gpsimd)
                        elif lc == 3:
                            dma_block(2, ots[2], nc.scalar)
                            dma_block(3, ots[3], nc.sync)

    nc.compile()
    return nc


def _host_prep(positives, mask, item_emb, pos_emb, Wz, Wv, Wq, Wk,
               gamma_q, beta_q, gamma_k, beta_k, sparse_w, gumbel):
    """Host-side constant folding + input staging + per-core shards."""
    f32 = np.float32
    bf16 = ml_dtypes.bfloat16
    positives = np.asarray(positives)
    maskf = np.asarray(mask).astype(f32)
    item_emb = np.asarray(item_emb, f32)
    pos_emb = np.asarray(pos_emb, f32)
    sw = np.asarray(sparse_w, f32)
    gum = np.asarray(gumbel, f32)

    smask = (1.0 / (1.0 + np.exp(-((np.log(sw / (1.0 - sw)) + gum) / f32(TEMP)))))
    smask = smask.astype(f32)
    scl = f32(1.0 / np.sqrt(L * H))
    j = np.arange(L)
    strict_lower_T = (j[:, None] < j[None, :])  # [j, l] : j < l
    M1s_mat = (smask.T * strict_lower_T * scl).astype(f32)
    dsv = (np.diag(smask) * scl).astype(f32)

    # one [128, CST_COLS] tile: causal-compacted M1s chunks + diag blocks
    cstp = np.zeros((P, CST_COLS), f32)
    for c in range(NC_CHUNKS):
        cstp[:, COFF[c]:COFF[c] + CW[c]] = M1s_mat[c * P:(c + 1) * P, c * P:]
        blk = np.zeros((P, P), f32)
        np.fill_diagonal(blk, dsv[c * P:(c + 1) * P])
        cstp[:, DOFF + c * P:DOFF + (c + 1) * P] = blk
    cstp = np.ascontiguousarray(cstp.astype(bf16))

    # Q@K^T folds to Z (Wq^T diag(gq*gk) Wk) Z^T only when both betas vanish
    # (true for this model's inputs); fail loudly rather than silently wrong.
    assert not np.any(np.asarray(beta_q)) and not np.any(np.asarray(beta_k)), (
        "kernel assumes beta_q == beta_k == 0 (holds for setup_inputs)"
    )
    g = np.asarray(gamma_q, np.float64) * np.asarray(gamma_k, np.float64)
    Wqk = (np.asarray(Wq, np.float64).T
           @ (g[:, None] * np.asarray(Wk, np.float64))).astype(f32)

    # host staging: XT_b = (emb[pos_b] + pos_emb)^T, chunked [4,128,L] bf16
    X = item_emb[positives] + pos_emb[None]                  # [B, L, H] f32
    XT = np.ascontiguousarray(X.transpose(0, 2, 1)).astype(bf16)
    XT = XT.reshape(B, NC_CHUNKS, P, L)

    # mask packed [128, BPC*4]: mskp[p, b*4+c] = mask[b, c*128+p]
    m4 = maskf.reshape(B, NC_CHUNKS, P)

    shared = {
        "WzT": np.ascontiguousarray(np.asarray(Wz, f32).T.astype(bf16)),
        "WvT": np.ascontiguousarray(np.asarray(Wv, f32).T.astype(bf16)),
        "Wqk": np.ascontiguousarray(Wqk.astype(bf16)),
        "cst": cstp,
    }
    in_maps = []
    for c in range(N_CORES):
        sl = slice(c * BPC, (c + 1) * BPC)
        m = dict(shared)
        m["XT"] = np.ascontiguousarray(XT[sl])
        m["mskp"] = np.ascontiguousarray(
            m4[sl].transpose(2, 0, 1).reshape(P, BPC * NC_CHUNKS)
        )
        in_maps.append(m)
    return in_maps


def get_module():
    global _COMPILED
    if _COMPILED is None:
        _COMPILED = _build_module()
    return _COMPILED


def kernel(**inputs) -> np.ndarray:
    nc = get_module()
    in_maps = _host_prep(**inputs)
    res = run_bass_kernel_spmd(nc, in_maps, core_ids=list(range(N_CORES)))
    out = np.concatenate([r["out"] for r in res.results], axis=0)
    return out.astype(np.float32)


if __name__ == "__main__":
    rng = np.random.default_rng(0)
    demo = {
        "positives": rng.integers(0, ITEM, (B, L)).astype(np.int32),
        "mask": rng.integers(0, 2, (B, L)).astype(np.int32),
        "item_emb": rng.normal(size=(ITEM, H)).astype(np.float32) * 0.02,
        "pos_emb": rng.normal(size=(L, H)).astype(np.float32) * 0.02,
        "Wz": rng.normal(size=(L, L)).astype(np.float32),
        "Wv": rng.normal(size=(L, L)).astype(np.float32),
        "Wq": rng.normal(size=(L, L)).astype(np.float32),
        "Wk": rng.normal(size=(L, L)).astype(np.float32),
        "gamma_q": rng.normal(size=(L,)).astype(np.float32) * 0.02,
        "beta_q": np.zeros((L,), np.float32),
        "gamma_k": rng.normal(size=(L,)).astype(np.float32) * 0.02,
        "beta_k": np.zeros((L,), np.float32),
        "sparse_w": rng.uniform(0.2, 0.8, (L, H)).astype(np.float32),
        "gumbel": rng.normal(size=(L, H)).astype(np.float32),
    }
    out = kernel(**demo)
    print("out", out.shape, out.dtype, np.abs(out).max())
